# revision 1
# baseline (speedup 1.0000x reference)
"""Trainium2 Bass kernel for a 2-layer ViT (local banded MHA + global MHA, CLS head).

Contract: kernel(**inputs) takes the FULL fp32 inputs (as produced by
setup_inputs()) and returns the FULL [64, 1000] fp32 output. Internally the
batch (64) is sharded 8-ways across NeuronCores (data parallel); parameters are
replicated. Self-contained: shapes/sharding hardcoded.

Math notes:
 - activations held TRANSPOSED on chip: [D=768 (6 x 128 partitions), Ntok]
   with the 257 tokens padded to 264 columns (pads are masked/ignored).
 - local banded attention (radius 1): scores computed as S^T[k, q] per
   128-token k-chunk against a 130-wide q window around the diagonal; the
   attention-value matmul accumulates the overlapping q-windows into one PSUM
   tile via the per-element has_written accumulate semantics.
 - softmax normalization is exact per head: column-sums of exp come from
   ones-vector matmuls (free via has_written accumulation), reciprocals are
   broadcast back across partitions with rank-1 matmuls, and the divide is
   fused into the PSUM eviction multiply.
 - layer-2 computes K/V for all tokens but Q/attention/output only for the
   CLS token (the only row the model head consumes).
 - weights/activations bf16 on-chip, accumulation fp32 in PSUM, LN stats fp32.
"""

import numpy as np
import ml_dtypes
from contextlib import ExitStack

BF16 = ml_dtypes.bfloat16

B, NCORES, BPC = 64, 8, 8
IMAGE, PATCH, GRID = 224, 14, 16
NPATCH, N, NPAD = 256, 257, 264
D, NH, HD, E, NCLS = 768, 12, 64, 2304, 1000
DC = D // 128            # 6 d-chunks
KP, KC = 98, 2           # patch-pixel contraction chunks: 196 = 2*98
SCALE = 1.0 / np.sqrt(HD)
NEG = -1e30
# k-chunks over tokens: (0:128, 128:256, 256:264); q-window per k-chunk
KCH = [(0, 128), (128, 128), (256, 8)]
QWIN = [(0, 130), (127, 130), (255, 9)]

_CACHE = {}


def _indh():
    ind = np.zeros((DC, 128, NH), np.float32)
    for dc in range(DC):
        for p in range(128):
            ind[dc, p, (128 * dc + p) // HD] = 1.0
    return ind.astype(BF16)


def _masks():
    m = np.full((3, 128, 130), NEG, np.float32)
    for c, ((k0, kn), (q0, qn)) in enumerate(zip(KCH, QWIN)):
        for kl in range(kn):
            gk = k0 + kl
            if gk > 256:
                continue
            for j in range(qn):
                gq = q0 + j
                if abs(gk - gq) <= 1 or (gq > 256 and gk <= 256):
                    m[c, kl, j] = 0.0
    return m


def build_nc(debug=False):
    import concourse.bacc as bacc
    import concourse.tile as tile
    from concourse import mybir
    import concourse.bass as bass

    f32, bf16 = mybir.dt.float32, mybir.dt.bfloat16
    AF, ALU = mybir.ActivationFunctionType, mybir.AluOpType

    nc = bacc.Bacc("TRN2", target_bir_lowering=False, debug=False)

    # ---- DRAM I/O ----
    d_pt = nc.dram_tensor("patchesT", [BPC, KC, KP, NPAD], bf16, kind="ExternalInput")
    d_wpT = nc.dram_tensor("wpT", [KP, KC, D], bf16, kind="ExternalInput")
    d_bp = nc.dram_tensor("bp", [DC, 128, NPAD], f32, kind="ExternalInput")
    d_wqkvT_l = nc.dram_tensor("wqkvT_l", [DC, 128, E], bf16, kind="ExternalInput")
    d_woT_l = nc.dram_tensor("woT_l", [DC, 128, D], bf16, kind="ExternalInput")
    d_wqkvT_g = nc.dram_tensor("wqkvT_g", [DC, 128, E], bf16, kind="ExternalInput")
    d_woT_g = nc.dram_tensor("woT_g", [DC, 128, D], bf16, kind="ExternalInput")
    d_wclsT = nc.dram_tensor("wclsT", [DC, 128, NCLS], bf16, kind="ExternalInput")
    d_mask = nc.dram_tensor("maskp", [3, 128, 130], f32, kind="ExternalInput")
    d_ident = nc.dram_tensor("ident", [128, 128], bf16, kind="ExternalInput")
    d_indh = nc.dram_tensor("indh", [DC, 128, NH], bf16, kind="ExternalInput")
    d_bqkv_l = nc.dram_tensor("bqkv_l", [E], f32, kind="ExternalInput")
    d_bo_l = nc.dram_tensor("bo_l", [D], f32, kind="ExternalInput")
    d_bqkv_g = nc.dram_tensor("bqkv_g", [E], f32, kind="ExternalInput")
    d_bo_g = nc.dram_tensor("bo_g", [D], f32, kind="ExternalInput")
    d_g1 = nc.dram_tensor("g1", [D], f32, kind="ExternalInput")
    d_be1 = nc.dram_tensor("be1", [D], f32, kind="ExternalInput")
    d_g2 = nc.dram_tensor("g2", [D], f32, kind="ExternalInput")
    d_be2 = nc.dram_tensor("be2", [D], f32, kind="ExternalInput")
    d_bcls = nc.dram_tensor("b_cls", [NCLS], f32, kind="ExternalInput")
    d_out = nc.dram_tensor("logits", [BPC, NCLS], f32, kind="ExternalOutput")
    dbg = {}
    if debug:
        for nm, shp in [("dbg_tok", [DC, 128, NPAD]), ("dbg_qk", [12, 128, NPAD]),
                        ("dbg_av", [DC, 128, NPAD]), ("dbg_x1", [DC, 128, NPAD]),
                        ("dbg_local", [DC, 128, NPAD]), ("dbg_kg", [DC, 128, NPAD]),
                        ("dbg_sg", [1, NPAD]), ("dbg_ag", [1, D])]:
            dbg[nm] = nc.dram_tensor(nm, shp, f32, kind="ExternalOutput")

    with tile.TileContext(nc) as tc, ExitStack() as ctx:
        konst = ctx.enter_context(tc.tile_pool(name="konst", bufs=1))
        acts = ctx.enter_context(tc.tile_pool(name="acts", bufs=2))
        small = ctx.enter_context(tc.tile_pool(name="small", bufs=4))
        ps_mm = ctx.enter_context(tc.tile_pool(name="ps_mm", bufs=3, space="PSUM"))
        ps_pair = ctx.enter_context(tc.tile_pool(name="ps_pair", bufs=2, space="PSUM"))
        ps_v = ctx.enter_context(tc.tile_pool(name="ps_v", bufs=1, space="PSUM"))
        ps_row = ctx.enter_context(tc.tile_pool(name="ps_row", bufs=1, space="PSUM"))

        # ---- persistent SBUF ----
        wpT = konst.tile([KP, KC, D], bf16)
        nc.sync.dma_start(wpT, d_wpT.ap())
        wqkv_l = konst.tile([128, DC, E], bf16)
        wo_l = konst.tile([128, DC, D], bf16)
        wqkv_g = konst.tile([128, DC, E], bf16)
        wo_g = konst.tile([128, DC, D], bf16)
        wcls = konst.tile([128, DC, NCLS], bf16)
        bp = konst.tile([128, DC, NPAD], f32)
        for d in range(DC):
            nc.sync.dma_start(wqkv_l[:, d, :], d_wqkvT_l.ap()[d])
            nc.sync.dma_start(wo_l[:, d, :], d_woT_l.ap()[d])
            nc.sync.dma_start(wqkv_g[:, d, :], d_wqkvT_g.ap()[d])
            nc.sync.dma_start(wo_g[:, d, :], d_woT_g.ap()[d])
            nc.sync.dma_start(wcls[:, d, :], d_wclsT.ap()[d])
            nc.sync.dma_start(bp[:, d, :], d_bp.ap()[d])
        mask = konst.tile([128, 3, 130], f32)
        for c in range(3):
            nc.sync.dma_start(mask[:, c, :], d_mask.ap()[c])
        ident = konst.tile([128, 128], bf16)
        nc.sync.dma_start(ident, d_ident.ap())
        indh = konst.tile([128, DC, NH], bf16)
        for d in range(DC):
            nc.sync.dma_start(indh[:, d, :], d_indh.ap()[d])
        zrow768 = konst.tile([1, D], bf16)
        bqkv_l_c = konst.tile([128, 18], f32)
        nc.sync.dma_start(bqkv_l_c, d_bqkv_l.ap().rearrange("(j p) -> p j", p=128))
        bqkv_g_c = konst.tile([128, 18], f32)
        nc.sync.dma_start(bqkv_g_c, d_bqkv_g.ap().rearrange("(j p) -> p j", p=128))
        bo_l_c = konst.tile([128, DC], f32)
        nc.sync.dma_start(bo_l_c, d_bo_l.ap().rearrange("(j p) -> p j", p=128))
        bo_g_c = konst.tile([128, DC], f32)
        nc.sync.dma_start(bo_g_c, d_bo_g.ap().rearrange("(j p) -> p j", p=128))
        g1_c = konst.tile([128, DC], f32)
        nc.sync.dma_start(g1_c, d_g1.ap().rearrange("(j p) -> p j", p=128))
        be1_c = konst.tile([128, DC], f32)
        nc.sync.dma_start(be1_c, d_be1.ap().rearrange("(j p) -> p j", p=128))
        g2_c = konst.tile([128, DC], f32)
        nc.sync.dma_start(g2_c, d_g2.ap().rearrange("(j p) -> p j", p=128))
        be2_c = konst.tile([128, DC], f32)
        nc.sync.dma_start(be2_c, d_be2.ap().rearrange("(j p) -> p j", p=128))
        bcls_r = konst.tile([BPC, NCLS], f32)
        nc.sync.dma_start(
            bcls_r,
            bass.AP(tensor=d_bcls, offset=0, ap=[[0, BPC], [1, NCLS]]),
        )
        ones_col = konst.tile([128, 1], bf16)
        nc.vector.memset(ones_col, 1.0)
        ones_row = konst.tile([1, 128], bf16)
        nc.vector.memset(ones_row, 1.0)
        zrow = konst.tile([1, NPAD], bf16)
        nc.vector.memset(zrow, 0.0)
        nc.vector.memset(zrow768, 0.0)
        epsc = konst.tile([1, 1], f32)
        nc.vector.memset(epsc, 1e-5)

        LOCAL = konst.tile([128, BPC, DC, NPAD], bf16)   # post-LN1, all batches
        AGROWS = konst.tile([BPC, D], bf16)              # global attn out rows
        QCLS = konst.tile([128, DC, BPC], f32)          # global q for CLS

        def evict(dst, src, bias=None, scale=1.0):
            if bias is None:
                nc.scalar.activation(dst, src, AF.Copy, scale=scale)
            else:
                nc.scalar.activation(dst, src, AF.Identity, bias=bias, scale=scale)

        # ================= pass 1: per batch through LN1 =================
        for b in range(BPC):
            pt = acts.tile([KP, KC, NPAD], bf16, tag="pt")
            for k in range(KC):
                nc.sync.dma_start(pt[:, k, :], d_pt.ap()[b, k])
            tokT = acts.tile([128, DC, NPAD], bf16, tag="tokT")
            for d in range(DC):
                ps = ps_mm.tile([128, NPAD], f32, tag="mm")
                for k in range(KC):
                    nc.tensor.matmul(ps, lhsT=wpT[:, k, bass.ts(d, 128)],
                                     rhs=pt[:, k, :], start=(k == 0), stop=(k == KC - 1))
                nc.vector.tensor_add(tokT[:, d, :], ps, bp[:, d, :])
            if debug and b == 0:
                for d in range(DC):
                    nc.gpsimd.dma_start(dbg["dbg_tok"].ap()[d], tokT[:, d, :])

            # Q^T,K^T (e-chunks 0..11), layer 1
            qkT = acts.tile([128, 12, NPAD], bf16, tag="qkT")
            for e in range(12):
                ps = ps_mm.tile([128, NPAD], f32, tag="mm")
                for d in range(DC):
                    nc.tensor.matmul(ps, lhsT=wqkv_l[:, d, bass.ts(e, 128)],
                                     rhs=tokT[:, d, :], start=(d == 0), stop=(d == DC - 1))
                evict(qkT[:, e, :], ps, bias=bqkv_l_c[:, e:e + 1])
            if debug and b == 0:
                for e in range(12):
                    nc.gpsimd.dma_start(dbg["dbg_qk"].ap()[e], qkT[:, e, :])

            # V token-major, layer 1
            v0 = acts.tile([128, D], bf16, tag="v0")
            v1 = acts.tile([128, D], bf16, tag="v1")
            v2 = acts.tile([8, D], bf16, tag="v2")
            for t, vt in ((0, v0), (1, v1), (2, v2)):
                k0, kn = KCH[t]
                ps = ps_v.tile([128, D], f32, tag="vps")
                for n0, nn in ((0, 512), (512, 256)):
                    for d in range(DC):
                        nc.tensor.matmul(ps[:kn, n0:n0 + nn],
                                         lhsT=tokT[:, d, k0:k0 + kn],
                                         rhs=wqkv_l[:, d, 2 * D + n0:2 * D + n0 + nn],
                                         start=(d == 0), stop=(d == DC - 1))
                evict(vt, ps[:kn, :])

            # local banded attention (unnormalized)
            AVT = acts.tile([128, DC, NPAD], bf16, tag="AVT")
            for hp in range(6):          # head pairs
                pair = ps_pair.tile([128, NPAD], f32, tag="pair")
                normps = ps_pair.tile([128, NPAD], f32, tag="pair")
                # zero the pair tile with a dummy start=True matmul so every
                # following AV matmul is a pure has_written accumulate; order
                # of the accumulating matmuls is then irrelevant.
                nc.tensor.matmul(pair, lhsT=ones_row, rhs=zrow,
                                 start=True, stop=False, skip_group_check=True)
                for hh in range(2):
                    h = 2 * hp + hh
                    r0 = 64 * hh
                    qh = qkT[r0:r0 + 64, h // 2, :]
                    kh = qkT[r0:r0 + 64, 6 + h // 2, :]
                    zh = ps_mm.tile([1, NPAD], f32, tag="mm")
                    nc.tensor.matmul(zh, lhsT=ones_row[0:1, 0:1], rhs=zrow,
                                     start=True, stop=False, skip_group_check=True)
                    for c in range(3):
                        (k0, kn), (q0, qn) = KCH[c], QWIN[c]
                        sps = ps_mm.tile([128, NPAD], f32, tag="mm")
                        nc.tensor.matmul(sps[:kn, :qn], lhsT=kh[:, k0:k0 + kn],
                                         rhs=qh[:, q0:q0 + qn], start=True, stop=True)
                        et = small.tile([128, 130], bf16, tag="et")
                        nc.vector.scalar_tensor_tensor(
                            out=et[:kn, :qn], in0=sps[:kn, :qn], scalar=SCALE,
                            in1=mask[:kn, c, :qn], op0=ALU.mult, op1=ALU.add)
                        nc.scalar.activation(et[:kn, :qn], et[:kn, :qn], AF.Exp)
                        nc.tensor.matmul(pair[r0:r0 + 64, q0:q0 + qn],
                                         lhsT=vt_sel(v0, v1, v2, c)[:kn, h * HD:(h + 1) * HD],
                                         rhs=et[:kn, :qn],
                                         start=False, stop=(hh == 1 and c == 2),
                                         skip_group_check=True)
                        nc.tensor.matmul(zh[0:1, q0:q0 + qn],
                                         lhsT=ones_col[:kn, :], rhs=et[:kn, :qn],
                                         start=False, stop=(c == 2),
                                         skip_group_check=True)
                    rz = small.tile([1, NPAD], f32, tag="rowb")
                    nc.vector.reciprocal(rz, zh)
                    rzb = small.tile([1, NPAD], bf16, tag="rowa")
                    evict(rzb, rz)
                    nc.tensor.matmul(normps[r0:r0 + 64, :],
                                     lhsT=ones_row[0:1, 0:64], rhs=rzb,
                                     start=True, stop=True, skip_group_check=True)
                norm_sb = small.tile([128, NPAD], bf16, tag="normsb")
                evict(norm_sb, normps)
                nc.vector.tensor_mul(AVT[:, hp, :], pair, norm_sb)
            if debug and b == 0:
                for d in range(DC):
                    nc.gpsimd.dma_start(dbg["dbg_av"].ap()[d], AVT[:, d, :])

            # out-projection layer 1
            x1 = acts.tile([128, DC, NPAD], bf16, tag="tokT")
            for e in range(DC):
                ps = ps_mm.tile([128, NPAD], f32, tag="mm")
                for f in range(DC):
                    nc.tensor.matmul(ps, lhsT=wo_l[:, f, bass.ts(e, 128)],
                                     rhs=AVT[:, f, :], start=(f == 0), stop=(f == DC - 1))
                evict(x1[:, e, :], ps, bias=bo_l_c[:, e:e + 1])
            if debug and b == 0:
                for d in range(DC):
                    nc.gpsimd.dma_start(dbg["dbg_x1"].ap()[d], x1[:, d, :])

            # LayerNorm 1 (stats over partition dim via ones-matmul)
            ps_s = ps_row.tile([1, NPAD], f32, tag="row")
            ps_q = ps_mm.tile([1, NPAD], f32, tag="mm")
            sq = acts.tile([128, NPAD], bf16, tag="sq")
            for d in range(DC):
                nc.vector.tensor_mul(sq, x1[:, d, :], x1[:, d, :])
                nc.tensor.matmul(ps_s, lhsT=ones_col, rhs=x1[:, d, :],
                                 start=(d == 0), stop=(d == DC - 1))
                nc.tensor.matmul(ps_q, lhsT=ones_col, rhs=sq,
                                 start=(d == 0), stop=(d == DC - 1))
            mu = small.tile([1, NPAD], bf16, tag="rowa")
            evict(mu, ps_s, scale=1.0 / D)
            muf = small.tile([1, NPAD], f32, tag="rowb")
            evict(muf, ps_s, scale=1.0 / D)
            m2 = small.tile([1, NPAD], f32, tag="rowa")
            nc.vector.tensor_mul(m2, muf, muf)
            var = small.tile([1, NPAD], f32, tag="rowb")
            nc.vector.scalar_tensor_tensor(out=var, in0=ps_q, scalar=1.0 / D,
                                           in1=m2, op0=ALU.mult, op1=ALU.subtract)
            sd = small.tile([1, NPAD], f32, tag="rowa")
            nc.scalar.activation(sd, var, AF.Sqrt, bias=epsc)
            rstd = small.tile([1, NPAD], f32, tag="rowb")
            nc.vector.reciprocal(rstd, sd)
            rstd_b = small.tile([1, NPAD], bf16, tag="rowa")
            evict(rstd_b, rstd)
            bmu_ps = ps_mm.tile([128, NPAD], f32, tag="mm")
            nc.tensor.matmul(bmu_ps, lhsT=ones_row, rhs=mu, start=True, stop=True)
            bmu = acts.tile([128, NPAD], bf16, tag="bmu")
            evict(bmu, bmu_ps)
            brs_ps = ps_mm.tile([128, NPAD], f32, tag="mm")
            nc.tensor.matmul(brs_ps, lhsT=ones_row, rhs=rstd_b, start=True, stop=True)
            brs = acts.tile([128, NPAD], bf16, tag="brs")
            evict(brs, brs_ps)
            for d in range(DC):
                t1 = acts.tile([128, NPAD], bf16, tag="t1")
                nc.vector.tensor_sub(t1, x1[:, d, :], bmu)
                t2 = acts.tile([128, NPAD], bf16, tag="t2")
                nc.vector.tensor_mul(t2, t1, brs)
                nc.scalar.activation(LOCAL[:, b, d, :], t2, AF.Identity,
                                     bias=be1_c[:, d:d + 1], scale=g1_c[:, d:d + 1])
            if debug and b == 0:
                for d in range(DC):
                    nc.gpsimd.dma_start(dbg["dbg_local"].ap()[d], LOCAL[:, b, d, :])

        # ================= pass 2: global q for CLS (all batches) =================
        for e in range(DC):
            ps = ps_row.tile([128, BPC], f32, tag="row")
            for d in range(DC):
                nc.tensor.matmul(ps, lhsT=wqkv_g[:, d, bass.ts(e, 128)],
                                 rhs=LOCAL[:, :, d, 0], start=(d == 0), stop=(d == DC - 1))
            evict(QCLS[:, e, :], ps, bias=bqkv_g_c[:, e:e + 1])

        # ================= pass 3: global attention per batch =================
        for b in range(BPC):
            kgT = acts.tile([128, 6, NPAD], bf16, tag="AVT")
            for e in range(DC):
                ps = ps_mm.tile([128, NPAD], f32, tag="mm")
                for d in range(DC):
                    nc.tensor.matmul(ps, lhsT=wqkv_g[:, d, D + 128 * e:D + 128 * (e + 1)],
                                     rhs=LOCAL[:, b, d, :], start=(d == 0), stop=(d == DC - 1))
                evict(kgT[:, e, :], ps, bias=bqkv_g_c[:, 6 + e:7 + e])
            if debug and b == 0:
                for d in range(DC):
                    nc.gpsimd.dma_start(dbg["dbg_kg"].ap()[d], kgT[:, d, :])
            vg0 = acts.tile([128, D], bf16, tag="v0")
            vg1 = acts.tile([128, D], bf16, tag="v1")
            vg2 = acts.tile([8, D], bf16, tag="v2")
            for t, vt in ((0, vg0), (1, vg1), (2, vg2)):
                k0, kn = KCH[t]
                ps = ps_v.tile([128, D], f32, tag="vps")
                for n0, nn in ((0, 512), (512, 256)):
                    for d in range(DC):
                        nc.tensor.matmul(ps[:kn, n0:n0 + nn],
                                         lhsT=LOCAL[:, b, d, k0:k0 + kn],
                                         rhs=wqkv_g[:, d, 2 * D + n0:2 * D + n0 + nn],
                                         start=(d == 0), stop=(d == DC - 1))
                evict(vt, ps[:kn, :])

            # per-head scores as columns: SCOL[k, h] per k-chunk via the
            # block-diagonal-expanded q (Qblk[d, h] = q[d] iff d in head h).
            qblk = small.tile([128, DC, NH], bf16, tag="qblk")
            for d in range(DC):
                nc.vector.tensor_scalar_mul(qblk[:, d, :], indh[:, d, :],
                                            QCLS[:, d, b:b + 1])
            scps = ps_mm.tile([128, 3 * NH], f32, tag="mm")
            nc.tensor.matmul(scps, lhsT=ones_row, rhs=zrow[:, 0:3 * NH],
                             start=True, stop=False, skip_group_check=True)
            for c in range(3):
                k0, kn = KCH[c]
                for d in range(DC):
                    nc.tensor.matmul(scps[:kn, NH * c:NH * (c + 1)],
                                     lhsT=kgT[:, d, k0:k0 + kn], rhs=qblk[:, d, :],
                                     start=False, stop=(c == 2 and d == DC - 1),
                                     skip_group_check=True)
            ecol = small.tile([128, 3 * NH], bf16, tag="ecol")
            # chunk 2 holds pad k-tokens 257..263 in rows 1..7: zero the block
            # first, then exp only the real row 0 (WAW keeps the order).
            nc.vector.memset(ecol[0:8, 2 * NH:3 * NH], 0.0)
            for c in range(3):
                kn = KCH[c][1] if c < 2 else 1
                nc.scalar.activation(ecol[:kn, NH * c:NH * (c + 1)],
                                     scps[:kn, NH * c:NH * (c + 1)], AF.Exp, scale=SCALE)
            zg = ps_mm.tile([1, NH], f32, tag="mm")
            for c in range(3):
                kn = KCH[c][1] if c < 2 else 1
                nc.tensor.matmul(zg, lhsT=ones_col[:kn, :],
                                 rhs=ecol[:kn, NH * c:NH * (c + 1)],
                                 start=(c == 0), stop=(c == 2), skip_group_check=True)
            rzg = small.tile([1, NH], f32, tag="rowb")
            nc.vector.reciprocal(rzg, zg)
            rzgb = small.tile([1, NH], bf16, tag="rowa")
            evict(rzgb, rzg)
            bzps = ps_mm.tile([128, NH], f32, tag="mm")
            nc.tensor.matmul(bzps, lhsT=ones_row, rhs=rzgb, start=True, stop=True)
            rzbc = small.tile([128, NH], bf16, tag="rzbc")
            evict(rzbc, bzps)
            ecoln = small.tile([128, 3 * NH], bf16, tag="ecoln")
            for c in range(3):
                kn = 128 if c < 2 else 8
                nc.vector.tensor_mul(ecoln[:kn, NH * c:NH * (c + 1)],
                                     ecol[:kn, NH * c:NH * (c + 1)], rzbc[:kn, :])

            agps = ps_v.tile([1, D], f32, tag="vps")
            for n0, nn in ((0, 512), (512, 256)):
                nc.tensor.matmul(agps[0:1, n0:n0 + nn], lhsT=ones_row[0:1, 0:1],
                                 rhs=zrow768[:, n0:n0 + nn],
                                 start=True, stop=False, skip_group_check=True)
            for h in range(NH):
                for c, vt in ((0, vg0), (1, vg1), (2, vg2)):
                    kn = KCH[c][1]
                    nc.tensor.matmul(agps[0:1, h * HD:(h + 1) * HD],
                                     lhsT=ecoln[:kn, NH * c + h:NH * c + h + 1],
                                     rhs=vt[:kn, h * HD:(h + 1) * HD],
                                     start=False, stop=(h == NH - 1 and c == 2),
                                     skip_group_check=True)
            if debug and b == 0:
                ag_sb = konst.tile([1, D], f32)
                evict(ag_sb, agps)
                nc.gpsimd.dma_start(dbg["dbg_ag"].ap(), ag_sb)
            agrow = small.tile([1, D], bf16, tag="rowa")
            evict(agrow, agps)
            nc.sync.dma_start(AGROWS[b:b + 1, :], agrow)

        # ================= tail: wo_g, LN2, classifier =================
        attg = konst.tile([128, DC, BPC], bf16)
        for d in range(DC):
            tps = ps_mm.tile([128, BPC], bf16, tag="mm")
            nc.tensor.transpose(tps, AGROWS[:, bass.ts(d, 128)], ident[0:BPC, 0:BPC])
            evict(attg[:, d, :], tps)
        ogt = konst.tile([128, DC, BPC], bf16)
        for e in range(DC):
            ps = ps_row.tile([128, BPC], f32, tag="row")
            for f in range(DC):
                nc.tensor.matmul(ps, lhsT=wo_g[:, f, bass.ts(e, 128)],
                                 rhs=attg[:, f, :], start=(f == 0), stop=(f == DC - 1))
            evict(ogt[:, e, :], ps, bias=bo_g_c[:, e:e + 1])
        # LN2
        ps_s = ps_row.tile([1, BPC], f32, tag="row")
        ps_q = ps_mm.tile([1, BPC], f32, tag="mm")
        for d in range(DC):
            sq2 = small.tile([128, BPC], bf16, tag="sq2")
            nc.vector.tensor_mul(sq2, ogt[:, d, :], ogt[:, d, :])
            nc.tensor.matmul(ps_s, lhsT=ones_col, rhs=ogt[:, d, :],
                             start=(d == 0), stop=(d == DC - 1))
            nc.tensor.matmul(ps_q, lhsT=ones_col, rhs=sq2,
                             start=(d == 0), stop=(d == DC - 1))
        mu = small.tile([1, BPC], bf16, tag="rowa")
        evict(mu, ps_s, scale=1.0 / D)
        muf = small.tile([1, BPC], f32, tag="rowb")
        evict(muf, ps_s, scale=1.0 / D)
        m2 = small.tile([1, BPC], f32, tag="rowa")
        nc.vector.tensor_mul(m2, muf, muf)
        var = small.tile([1, BPC], f32, tag="rowb")
        nc.vector.scalar_tensor_tensor(out=var, in0=ps_q, scalar=1.0 / D,
                                       in1=m2, op0=ALU.mult, op1=ALU.subtract)
        sd = small.tile([1, BPC], f32, tag="rowa")
        nc.scalar.activation(sd, var, AF.Sqrt, bias=epsc)
        rstd = small.tile([1, BPC], f32, tag="rowb")
        nc.vector.reciprocal(rstd, sd)
        rstd_b = small.tile([1, BPC], bf16, tag="rowa")
        evict(rstd_b, rstd)
        bmu = ps_mm.tile([128, BPC], f32, tag="mm")
        nc.tensor.matmul(bmu, lhsT=ones_row, rhs=mu, start=True, stop=True)
        brs = ps_mm.tile([128, BPC], f32, tag="mm")
        nc.tensor.matmul(brs, lhsT=ones_row, rhs=rstd_b, start=True, stop=True)
        lng = konst.tile([128, DC, BPC], bf16)
        for d in range(DC):
            t1 = small.tile([128, BPC], bf16, tag="t1s")
            nc.vector.tensor_sub(t1, ogt[:, d, :], bmu)
            t2 = small.tile([128, BPC], bf16, tag="t2s")
            nc.vector.tensor_mul(t2, t1, brs)
            nc.scalar.activation(lng[:, d, :], t2, AF.Identity,
                                 bias=be2_c[:, d:d + 1], scale=g2_c[:, d:d + 1])
        # classifier
        outsb = konst.tile([BPC, NCLS], f32)
        for n0, nn in ((0, 512), (512, NCLS - 512)):
            ps = ps_mm.tile([BPC, 512], f32, tag="mm")
            for d in range(DC):
                nc.tensor.matmul(ps[:, :nn], lhsT=lng[:, d, :],
                                 rhs=wcls[:, d, n0:n0 + nn],
                                 start=(d == 0), stop=(d == DC - 1))
            nc.vector.tensor_add(outsb[:, n0:n0 + nn], ps[:, :nn], bcls_r[:, n0:n0 + nn])
        nc.sync.dma_start(d_out.ap(), outsb)

    nc.compile()
    return nc


def vt_sel(v0, v1, v2, c):
    return (v0, v1, v2)[c]


def prep_inputs(inputs):
    """numpy-only host prep: shard x; transpose/bcast/pack parameters."""
    f = lambda k: np.asarray(inputs[k], np.float32)
    x = f("x")
    pat = x[:, 0].reshape(B, GRID, PATCH, GRID, PATCH)
    pat = pat.transpose(0, 2, 4, 1, 3).reshape(B, PATCH * PATCH, NPATCH)
    patchesT = np.zeros((B, KP * KC, NPAD), np.float32)
    patchesT[:, :, 1:N] = pat
    patchesT = patchesT.reshape(B, KC, KP, NPAD).astype(BF16)

    wpT = f("w_patch").T.reshape(KC, KP, D).transpose(1, 0, 2).astype(BF16)

    pos = f("pos_embedding")[0]              # [257, 768]
    bp = np.zeros((D, NPAD), np.float32)
    bp[:, 1:N] = f("b_patch")[:, None] + pos[1:].T
    bp[:, 0] = f("cls_token")[0, 0] + pos[0]
    bp = bp.reshape(DC, 128, NPAD)

    shared = {
        "wpT": wpT,
        "bp": bp,
        "wqkvT_l": f("wqkv_l").T.reshape(DC, 128, E).astype(BF16),
        "woT_l": f("wo_l").T.reshape(DC, 128, D).astype(BF16),
        "wqkvT_g": f("wqkv_g").T.reshape(DC, 128, E).astype(BF16),
        "woT_g": f("wo_g").T.reshape(DC, 128, D).astype(BF16),
        "wclsT": f("w_cls").T.reshape(DC, 128, NCLS).astype(BF16),
        "maskp": _masks(),
        "ident": np.eye(128, dtype=np.float32).astype(BF16),
        "indh": _indh(),
        "bqkv_l": f("bqkv_l"), "bo_l": f("bo_l"),
        "bqkv_g": f("bqkv_g"), "bo_g": f("bo_g"),
        "g1": f("g1"), "be1": f("be1"), "g2": f("g2"), "be2": f("be2"),
        "b_cls": f("b_cls"),
    }
    in_maps = []
    for c in range(NCORES):
        m = dict(shared)
        m["patchesT"] = patchesT[c * BPC:(c + 1) * BPC]
        in_maps.append(m)
    return in_maps


def kernel(**inputs) -> np.ndarray:
    if "nc" not in _CACHE:
        _CACHE["nc"] = build_nc(debug=False)
    nc = _CACHE["nc"]
    from concourse.bass_utils import run_bass_kernel_spmd
    in_maps = prep_inputs(inputs)
    res = run_bass_kernel_spmd(nc, in_maps, core_ids=list(range(NCORES)))
    return np.concatenate([r["logits"] for r in res.results], axis=0).astype(np.float32)



# revision 9
# speedup vs baseline: 1.3516x; 1.3516x over previous
"""Trainium2 Bass kernel for a 2-layer ViT (local banded MHA + global MHA, CLS head).

Contract: kernel(**inputs) takes the FULL fp32 inputs (as produced by
setup_inputs()) and returns the FULL [64, 1000] fp32 output. Internally the
batch (64) is sharded 8-ways across NeuronCores (data parallel); parameters are
replicated. Self-contained: shapes/sharding hardcoded.

v2 design notes (vs the first working version):
 - activations transposed on chip: [D=768 (6 x 128 partitions), Ntok], tokens
   padded 257 -> 264 columns.
 - local banded attention (radius 1) per head: three k-major score grids land
   in one PSUM bank [128, 269]; one Exp (1/sqrt(hd) scale fused) evicts to
   bf16, one 0/1 band-mask multiply on DVE kills off-band entries. AV uses
   disjoint q-ranges per chunk plus 1-column boundary matmuls (contraction-64
   at base partition 64 where needed, relying on the mask zeros), so no PSUM
   zero-init matmuls and no -inf mask tensor.
 - softmax denominators: per-pair row sums at partitions {0,32} of one PSUM
   bank; one strided DVE reciprocal; a rank-1 "pairsel" matmul broadcasts the
   reciprocals across partitions; the divide fuses into the PSUM->SBUF
   eviction multiply.
 - the 257th token's V is computed as columns (36 1-col matmuls) and bounced
   through DRAM into row layout, avoiding full-width 768-col matmuls for an
   8-token chunk.
 - evictions split between Act and DVE; patch-embed bias adds and LN squares
   run on the idle GPSIMD (Pool) engine.
 - per-item work is software-pipelined (embed/qkv of item b+1 enqueued before
   attention of item b) so the tensor engine rarely waits on cross-engine
   softmax round trips.
 - layer 2 computes K/V for all tokens but Q/attention/output only for the
   CLS token; QCLS accumulates incrementally per item (no global barrier).
"""

import numpy as np
import ml_dtypes
from contextlib import ExitStack

BF16 = ml_dtypes.bfloat16

B, NCORES, BPC = 64, 8, 8
IMAGE, PATCH, GRID = 224, 14, 16
NPATCH, N, NPAD = 256, 257, 264
D, NH, HD, E, NCLS = 768, 12, 64, 2304, 1000
DC = D // 128             # 6 d-chunks
KP, KC = 98, 2            # patch-pixel contraction chunks: 196 = 2*98
SCALE = 1.0 / np.sqrt(HD)
SCW = 269                 # score grid cols: 130 (g0) + 130 (g1) + 9 (g2)

_CACHE = {}


def _indh():
    ind = np.zeros((DC, 128, NH), np.float32)
    for dc in range(DC):
        for p in range(128):
            ind[dc, p, (128 * dc + p) // HD] = 1.0
    return ind


def _band01():
    m = np.zeros((128, SCW), np.float32)
    for kl in range(128):
        for j in range(130):
            # g0: k = kl, q = j (j <= 128 real)
            if j <= 128 and abs(kl - j) <= 1:
                m[kl, j] = 1.0
            # g1: k = 128 + kl, q = 127 + j
            if abs(kl + 1 - j) <= 1:
                m[kl, 130 + j] = 1.0
    # g2: row 0 = k 256; cols j: q = 255 + j (boundary, q256, pads)
    for j in range(9):
        m[0, 260 + j] = 1.0
    return m.astype(BF16)


def build_nc(debug=False):
    import concourse.bacc as bacc
    import concourse.tile as tile
    from concourse import mybir
    import concourse.bass as bass

    f32, bf16 = mybir.dt.float32, mybir.dt.bfloat16
    AF, ALU = mybir.ActivationFunctionType, mybir.AluOpType

    nc = bacc.Bacc("TRN2", target_bir_lowering=False, debug=False)

    # ---- DRAM I/O ----
    d_pt = nc.dram_tensor("patchesT", [BPC, KC, KP, NPAD], bf16, kind="ExternalInput")
    d_wpT = nc.dram_tensor("wpT", [KP, KC, D], bf16, kind="ExternalInput")
    d_bp = nc.dram_tensor("bp", [DC, 128, NPAD], f32, kind="ExternalInput")
    d_wqkvT_l = nc.dram_tensor("wqkvT_l", [DC, 128, E], bf16, kind="ExternalInput")
    d_woT_l = nc.dram_tensor("woT_l", [DC, 128, D], bf16, kind="ExternalInput")
    d_wqkvT_g = nc.dram_tensor("wqkvT_g", [DC, 128, E], bf16, kind="ExternalInput")
    d_woT_g = nc.dram_tensor("woT_g", [DC, 128, D], bf16, kind="ExternalInput")
    d_wclsT = nc.dram_tensor("wclsT", [DC, 128, NCLS], bf16, kind="ExternalInput")
    d_band = nc.dram_tensor("band01", [128, SCW], bf16, kind="ExternalInput")
    d_psel = nc.dram_tensor("pairsel", [2, 128], bf16, kind="ExternalInput")
    d_ident = nc.dram_tensor("ident", [128, 128], bf16, kind="ExternalInput")
    d_indh = nc.dram_tensor("indh", [DC, 128, NH], bf16, kind="ExternalInput")
    d_biasblk = nc.dram_tensor("biasblk", [DC, 128, NH], bf16, kind="ExternalInput")
    d_bqkv_l = nc.dram_tensor("bqkv_l", [E], f32, kind="ExternalInput")
    d_bo_l = nc.dram_tensor("bo_l", [D], f32, kind="ExternalInput")
    d_bqkv_g = nc.dram_tensor("bqkv_g", [E], f32, kind="ExternalInput")
    d_bo_g = nc.dram_tensor("bo_g", [D], f32, kind="ExternalInput")
    d_g1 = nc.dram_tensor("g1", [D], f32, kind="ExternalInput")
    d_be1 = nc.dram_tensor("be1", [D], f32, kind="ExternalInput")
    d_g2 = nc.dram_tensor("g2", [D], f32, kind="ExternalInput")
    d_be2 = nc.dram_tensor("be2", [D], f32, kind="ExternalInput")
    d_bcls = nc.dram_tensor("b_cls", [NCLS], f32, kind="ExternalInput")
    d_v2scr = nc.dram_tensor("v2scr", [2, BPC, D], bf16, kind="Internal")
    d_out = nc.dram_tensor("logits", [BPC, NCLS], f32, kind="ExternalOutput")

    with tile.TileContext(nc) as tc, ExitStack() as ctx:
        konst = ctx.enter_context(tc.tile_pool(name="konst", bufs=1))
        acts = ctx.enter_context(tc.tile_pool(name="acts", bufs=2))
        small = ctx.enter_context(tc.tile_pool(name="small", bufs=2))
        # PSUM: 8 banks total: mm 2 + sc 2 + av(+zh) 2 + b1 2
        ps_mm = ctx.enter_context(tc.tile_pool(name="ps_mm", bufs=2, space="PSUM"))
        ps_sc = ctx.enter_context(tc.tile_pool(name="ps_sc", bufs=2, space="PSUM"))
        ps_av = ctx.enter_context(tc.tile_pool(name="ps_av", bufs=2, space="PSUM"))
        ps_b1 = ctx.enter_context(tc.tile_pool(name="ps_b1", bufs=2, space="PSUM"))

        # ---- persistent SBUF ----
        wpT = konst.tile([KP, KC, D], bf16)
        nc.sync.dma_start(wpT, d_wpT.ap())
        wqkv_l = konst.tile([128, DC, E], bf16)
        wo_l = konst.tile([128, DC, D], bf16)
        wqkv_g = konst.tile([128, DC, E], bf16)
        wo_g = konst.tile([128, DC, D], bf16)
        wcls = konst.tile([128, DC, NCLS], bf16)
        bp = konst.tile([128, DC, NPAD], f32)
        for d in range(DC):
            nc.sync.dma_start(wqkv_l[:, d, :], d_wqkvT_l.ap()[d])
            nc.sync.dma_start(wo_l[:, d, :], d_woT_l.ap()[d])
            nc.sync.dma_start(wqkv_g[:, d, :], d_wqkvT_g.ap()[d])
            nc.sync.dma_start(wo_g[:, d, :], d_woT_g.ap()[d])
            nc.sync.dma_start(wcls[:, d, :], d_wclsT.ap()[d])
            nc.sync.dma_start(bp[:, d, :], d_bp.ap()[d])
        band01 = konst.tile([128, SCW], bf16)
        nc.sync.dma_start(band01, d_band.ap())
        pairsel = konst.tile([2, 128], bf16)
        nc.sync.dma_start(pairsel, d_psel.ap())
        ident = konst.tile([128, 128], bf16)
        nc.sync.dma_start(ident, d_ident.ap())
        indh = konst.tile([128, DC, NH], bf16)
        biasblk = konst.tile([128, DC, NH], bf16)
        for d in range(DC):
            nc.sync.dma_start(indh[:, d, :], d_indh.ap()[d])
            nc.sync.dma_start(biasblk[:, d, :], d_biasblk.ap()[d])
        bqkv_l_c = konst.tile([128, 18], f32)
        nc.sync.dma_start(bqkv_l_c, d_bqkv_l.ap().rearrange("(j p) -> p j", p=128))
        bqkv_g_c = konst.tile([128, 18], f32)
        nc.sync.dma_start(bqkv_g_c, d_bqkv_g.ap().rearrange("(j p) -> p j", p=128))
        bo_l_c = konst.tile([128, DC], f32)
        nc.sync.dma_start(bo_l_c, d_bo_l.ap().rearrange("(j p) -> p j", p=128))
        bo_g_c = konst.tile([128, DC], f32)
        nc.sync.dma_start(bo_g_c, d_bo_g.ap().rearrange("(j p) -> p j", p=128))
        g1_c = konst.tile([128, DC], f32)
        nc.sync.dma_start(g1_c, d_g1.ap().rearrange("(j p) -> p j", p=128))
        be1_c = konst.tile([128, DC], f32)
        nc.sync.dma_start(be1_c, d_be1.ap().rearrange("(j p) -> p j", p=128))
        g2_c = konst.tile([128, DC], f32)
        nc.sync.dma_start(g2_c, d_g2.ap().rearrange("(j p) -> p j", p=128))
        be2_c = konst.tile([128, DC], f32)
        nc.sync.dma_start(be2_c, d_be2.ap().rearrange("(j p) -> p j", p=128))
        bcls_r = konst.tile([BPC, NCLS], f32)
        nc.sync.dma_start(
            bcls_r, bass.AP(tensor=d_bcls, offset=0, ap=[[0, BPC], [1, NCLS]])
        )
        ones_col = konst.tile([128, 1], bf16)
        nc.vector.memset(ones_col, 1.0)
        ones_row = konst.tile([1, 128], bf16)
        nc.vector.memset(ones_row, 1.0)
        epsc = konst.tile([1, 1], f32)
        nc.vector.memset(epsc, 1e-5)

        LOCAL = konst.tile([128, BPC, DC, NPAD], bf16)   # post-LN1, all batches
        AGROWS = konst.tile([BPC, D], bf16)              # global attn out rows
        QCLS = konst.tile([128, DC, BPC], f32)           # global q for CLS (no bias)

        def evict(dst, src, bias=None, scale=1.0):
            if bias is None:
                nc.scalar.activation(dst, src, AF.Copy, scale=scale)
            else:
                nc.scalar.activation(dst, src, AF.Identity, bias=bias, scale=scale)

        MM = nc.tensor.matmul

        # --------------- pass 1 stages ---------------
        def s_embed(b):
            """patch embed + pos bias -> tokT(b)"""
            pt = acts.tile([KP, KC, NPAD], bf16, tag="pt")
            for k in range(KC):
                nc.sync.dma_start(pt[:, k, :], d_pt.ap()[b, k])
            tokT = acts.tile([128, DC, NPAD], bf16, tag="tokT")
            for d in range(DC):
                ps = ps_mm.tile([128, NPAD], f32, tag="mm")
                for k in range(KC):
                    MM(ps, lhsT=wpT[:, k, bass.ts(d, 128)], rhs=pt[:, k, :],
                       start=(k == 0), stop=(k == KC - 1))
                nc.gpsimd.tensor_add(tokT[:, d, :], ps, bp[:, d, :])
            return tokT

        def s_qkv(b, tokT, wqkv, bias_c, layer):
            """Q^T,K^T e-chunks (layer 1 only); V token-major chunks 0/1;
            tail-token V row via DRAM bounce. Returns (qkT, v0, v1, v2row)."""
            qkT = None
            if layer == 0:
                qkT = acts.tile([128, 12, NPAD], bf16, tag="qkT")
                for e in range(12):
                    ps = ps_mm.tile([128, NPAD], f32, tag="mm")
                    for d in range(DC):
                        MM(ps, lhsT=wqkv[:, d, bass.ts(e, 128)], rhs=tokT[:, d, :],
                           start=(d == 0), stop=(d == DC - 1))
                    if e % 2 == 0:
                        evict(qkT[:, e, :], ps, bias=bias_c[:, e:e + 1])
                    else:
                        nc.vector.tensor_scalar_add(qkT[:, e, :], ps,
                                                    bias_c[:, e:e + 1])
            v0 = acts.tile([128, D], bf16, tag="v0")
            v1 = acts.tile([128, D], bf16, tag="v1")
            for t, vt in ((0, v0), (1, v1)):
                for n0, nn in ((0, 512), (512, 256)):
                    ps = ps_mm.tile([128, nn], f32, tag="mm")
                    for d in range(DC):
                        MM(ps, lhsT=tokT[:, d, bass.ts(t, 128)],
                           rhs=wqkv[:, d, 2 * D + n0:2 * D + n0 + nn],
                           start=(d == 0), stop=(d == DC - 1))
                    evict(vt[:, n0:n0 + nn], ps)
            # tail token (256): V as columns -> DRAM bounce -> row layout
            v2t = ps_b1.tile([128, DC], f32, tag="b1")
            nc.gpsimd.memset(v2t, 0.0)
            for c in range(DC):
                for d in range(DC):
                    MM(v2t[:, c:c + 1],
                       lhsT=wqkv[:, d, 2 * D + 128 * c:2 * D + 128 * (c + 1)],
                       rhs=tokT[:, d, 256:257],
                       start=False, stop=(c == DC - 1 and d == DC - 1),
                       skip_group_check=True)
            v2t_sb = small.tile([128, DC], bf16, tag="v2t")
            nc.vector.tensor_scalar_add(v2t_sb, v2t, 0.0)
            nc.gpsimd.dma_start(
                bass.AP(tensor=d_v2scr, offset=(layer * BPC + b) * D,
                        ap=[[1, 128], [128, DC]]), v2t_sb)
            v2row = small.tile([1, D], bf16, tag="v2row")
            nc.gpsimd.dma_start(
                v2row, bass.AP(tensor=d_v2scr, offset=(layer * BPC + b) * D,
                               ap=[[D, 1], [1, D]]))
            return qkT, v0, v1, v2row

        def s_attn(b, qkT, v0, v1, v2row):
            """local banded attention -> AVT(b) (normalized, bf16)."""
            et = acts.tile([128, NH, SCW], bf16, tag="et", bufs=1)
            AVT = acts.tile([128, DC, NPAD], bf16, tag="AVT")

            def sps_head(h):
                qh = qkT[64 * (h % 2):64 * (h % 2) + 64, h // 2, :]
                kh = qkT[64 * (h % 2):64 * (h % 2) + 64, 6 + h // 2, :]
                sc = ps_sc.tile([128, SCW], f32, tag="sc")
                MM(sc[:, 0:130], lhsT=kh[:, 0:128], rhs=qh[:, 0:130],
                   start=True, stop=False, skip_group_check=True)
                MM(sc[:, 130:260], lhsT=kh[:, 128:256], rhs=qh[:, 127:257],
                   start=True, stop=False, skip_group_check=True)
                MM(sc[0:1, 260:269], lhsT=kh[:, 256:257], rhs=qh[:, 255:264],
                   start=True, stop=True, skip_group_check=True)
                nc.vector.memset(sc[1:128, 260:269], 0.0)
                nc.scalar.activation(et[:, h, :], sc, AF.Exp, scale=SCALE)
                nc.vector.tensor_mul(et[:, h, :], et[:, h, :], band01)

            def z_head(zh, h, hh, last):
                r = 32 * hh
                MM(zh[r:r + 1, 0:128], lhsT=ones_col, rhs=et[:, h, 0:128],
                   start=False, stop=False, skip_group_check=True)
                MM(zh[r:r + 1, 128:256], lhsT=ones_col, rhs=et[:, h, 131:259],
                   start=False, stop=False, skip_group_check=True)
                MM(zh[r:r + 1, 256:264], lhsT=ones_col[0:1, :],
                   rhs=et[0:1, h, 261:269],
                   start=False, stop=False, skip_group_check=True)
                # boundary columns (contraction-32 reads rely on mask zeros)
                MM(zh[r:r + 1, 128:129], lhsT=ones_col[64:128, :],
                   rhs=et[64:128, h, 128:129],
                   start=False, stop=False, skip_group_check=True)
                MM(zh[r:r + 1, 127:128], lhsT=ones_col[0:1, :],
                   rhs=et[0:1, h, 130:131],
                   start=False, stop=False, skip_group_check=True)
                MM(zh[r:r + 1, 256:257], lhsT=ones_col[64:128, :],
                   rhs=et[64:128, h, 259:260],
                   start=False, stop=False, skip_group_check=True)
                MM(zh[r:r + 1, 255:256], lhsT=ones_col[0:1, :],
                   rhs=et[0:1, h, 260:261],
                   start=False, stop=last, skip_group_check=True)

            def av_head(av, h, hh, last):
                r0 = 64 * hh
                hs = slice(h * HD, (h + 1) * HD)
                v2r = v2row[0:1, hs]
                MM(av[r0:r0 + 64, 0:128], lhsT=v0[:, hs], rhs=et[:, h, 0:128],
                   start=False, stop=False, skip_group_check=True)
                MM(av[r0:r0 + 64, 128:256], lhsT=v1[:, hs], rhs=et[:, h, 131:259],
                   start=False, stop=False, skip_group_check=True)
                MM(av[r0:r0 + 64, 256:264], lhsT=v2r, rhs=et[0:1, h, 261:269],
                   start=False, stop=False, skip_group_check=True)
                MM(av[r0:r0 + 64, 128:129], lhsT=v0[64:128, hs],
                   rhs=et[64:128, h, 128:129],
                   start=False, stop=False, skip_group_check=True)
                MM(av[r0:r0 + 64, 127:128], lhsT=v1[0:1, hs],
                   rhs=et[0:1, h, 130:131],
                   start=False, stop=False, skip_group_check=True)
                MM(av[r0:r0 + 64, 256:257], lhsT=v1[64:128, hs],
                   rhs=et[64:128, h, 259:260],
                   start=False, stop=False, skip_group_check=True)
                MM(av[r0:r0 + 64, 255:256], lhsT=v2r, rhs=et[0:1, h, 260:261],
                   start=False, stop=last, skip_group_check=True)

            sps_head(0)
            sps_head(1)
            state = {}
            for dc in range(DC):
                zh = ps_av.tile([33, 512], f32, tag="av")
                nc.gpsimd.memset(zh[0:33:32, 0:NPAD], 0.0)
                z_head(zh, 2 * dc, 0, False)
                z_head(zh, 2 * dc + 1, 1, True)
                rz2 = small.tile([2, NPAD], bf16, tag="rz2")
                with nc.allow_low_precision(reason="softmax denom, bf16 ok"):
                    nc.vector.reciprocal(rz2, zh[0:33:32, 0:NPAD])
                if dc < DC - 1:
                    sps_head(2 * dc + 2)
                    sps_head(2 * dc + 3)
                av = ps_av.tile([128, NPAD], f32, tag="av")
                nc.gpsimd.memset(av, 0.0)
                av_head(av, 2 * dc, 0, False)
                av_head(av, 2 * dc + 1, 1, True)
                nps = ps_b1.tile([128, NPAD], f32, tag="b1")
                MM(nps, lhsT=pairsel, rhs=rz2, start=True, stop=True)
                nc.vector.tensor_mul(AVT[:, dc, :], av, nps)
            return AVT

        def s_out_ln(b, AVT):
            """out-proj + LayerNorm1 -> LOCAL[:, b]; QCLS contribution."""
            x1 = acts.tile([128, DC, NPAD], bf16, tag="x1")
            for e in range(DC):
                ps = ps_mm.tile([128, NPAD], f32, tag="mm")
                for f in range(DC):
                    MM(ps, lhsT=wo_l[:, f, bass.ts(e, 128)], rhs=AVT[:, f, :],
                       start=(f == 0), stop=(f == DC - 1))
                if e % 2 == 0:
                    evict(x1[:, e, :], ps, bias=bo_l_c[:, e:e + 1])
                else:
                    nc.vector.tensor_scalar_add(x1[:, e, :], ps, bo_l_c[:, e:e + 1])
            # stats: rows 0 (sum x) and 32 (sum x^2) of one PSUM tile
            st = ps_b1.tile([33, 512], f32, tag="b1")
            sq = acts.tile([128, NPAD], bf16, tag="sq")
            for d in range(DC):
                nc.gpsimd.tensor_mul(sq, x1[:, d, :], x1[:, d, :])
                MM(st[0:1, 0:NPAD], lhsT=ones_col, rhs=x1[:, d, :],
                   start=(d == 0), stop=False, skip_group_check=True)
                MM(st[32:33, 0:NPAD], lhsT=ones_col, rhs=sq,
                   start=(d == 0), stop=(d == DC - 1), skip_group_check=True)
            murow = small.tile([1, NPAD], bf16, tag="mu")
            nc.vector.tensor_scalar_mul(murow, st[0:1, 0:NPAD], 1.0 / D)
            m2 = small.tile([1, NPAD], f32, tag="m2")
            nc.vector.tensor_mul(m2, murow, murow)
            varf = small.tile([1, NPAD], f32, tag="varf")
            nc.vector.scalar_tensor_tensor(out=varf, in0=st[32:33, 0:NPAD], scalar=1.0 / D,
                                           in1=m2, op0=ALU.mult, op1=ALU.subtract)
            sd = small.tile([1, NPAD], f32, tag="sd")
            nc.scalar.activation(sd, varf, AF.Sqrt, bias=epsc)
            rstd = small.tile([1, NPAD], bf16, tag="rstd")
            with nc.allow_low_precision(reason="rstd bf16 ok"):
                nc.vector.reciprocal(rstd, sd)
            bmu_ps = ps_mm.tile([128, NPAD], f32, tag="mm")
            MM(bmu_ps, lhsT=ones_row, rhs=murow, start=True, stop=True)
            bmu = acts.tile([128, NPAD], bf16, tag="bmu")
            evict(bmu, bmu_ps)
            brs_ps = ps_mm.tile([128, NPAD], f32, tag="mm")
            MM(brs_ps, lhsT=ones_row, rhs=rstd, start=True, stop=True)
            brs = acts.tile([128, NPAD], bf16, tag="brs")
            evict(brs, brs_ps)
            for d in range(DC):
                t1 = acts.tile([128, NPAD], bf16, tag="t1")
                nc.vector.tensor_sub(t1, x1[:, d, :], bmu)
                t2 = acts.tile([128, NPAD], bf16, tag="t2")
                nc.vector.tensor_mul(t2, t1, brs)
                nc.scalar.activation(LOCAL[:, b, d, :], t2, AF.Identity,
                                     bias=be1_c[:, d:d + 1], scale=g1_c[:, d:d + 1])
            # QCLS contribution for this item (global q for CLS, bias-free)
            qc = ps_b1.tile([128, DC], f32, tag="b1")
            nc.gpsimd.memset(qc, 0.0)
            for e in range(DC):
                for d in range(DC):
                    MM(qc[:, e:e + 1], lhsT=wqkv_g[:, d, bass.ts(e, 128)],
                       rhs=LOCAL[:, b, d, 0:1],
                       start=False, stop=(e == DC - 1 and d == DC - 1),
                       skip_group_check=True)
            nc.vector.tensor_scalar_add(QCLS[:, :, b], qc, 0.0)

        # --------------- pass 1: software-pipelined emission ---------------
        stage1 = {}
        def emit_front(b):
            tokT = s_embed(b)
            stage1[b] = s_qkv(b, tokT, wqkv_l, bqkv_l_c, 0)
        emit_front(0)
        emit_front(1)
        for b in range(BPC):
            AVT = s_attn(b, *stage1.pop(b))
            if b + 2 < BPC:
                emit_front(b + 2)
            s_out_ln(b, AVT)

        # --------------- pass 3 stages (global attention) ---------------
        def g_kv(b):
            kgT = acts.tile([128, DC, NPAD], bf16, tag="kgT")
            for e in range(DC):
                ps = ps_mm.tile([128, NPAD], f32, tag="mm")
                for d in range(DC):
                    MM(ps, lhsT=wqkv_g[:, d, D + 128 * e:D + 128 * (e + 1)],
                       rhs=LOCAL[:, b, d, :], start=(d == 0), stop=(d == DC - 1))
                if e % 2 == 0:
                    evict(kgT[:, e, :], ps, bias=bqkv_g_c[:, 6 + e:7 + e])
                else:
                    nc.vector.tensor_scalar_add(kgT[:, e, :], ps,
                                                bqkv_g_c[:, 6 + e:7 + e])
            vg0 = acts.tile([128, D], bf16, tag="v0")
            vg1 = acts.tile([128, D], bf16, tag="v1")
            for t, vt in ((0, vg0), (1, vg1)):
                for n0, nn in ((0, 512), (512, 256)):
                    ps = ps_mm.tile([128, nn], f32, tag="mm")
                    for d in range(DC):
                        MM(ps, lhsT=LOCAL[:, b, d, bass.ts(t, 128)],
                           rhs=wqkv_g[:, d, 2 * D + n0:2 * D + n0 + nn],
                           start=(d == 0), stop=(d == DC - 1))
                    evict(vt[:, n0:n0 + nn], ps)
            vg2t = ps_b1.tile([128, DC], f32, tag="b1")
            nc.gpsimd.memset(vg2t, 0.0)
            for c in range(DC):
                for d in range(DC):
                    MM(vg2t[:, c:c + 1],
                       lhsT=wqkv_g[:, d, 2 * D + 128 * c:2 * D + 128 * (c + 1)],
                       rhs=LOCAL[:, b, d, 256:257],
                       start=False, stop=(c == DC - 1 and d == DC - 1),
                       skip_group_check=True)
            vg2t_sb = small.tile([128, DC], bf16, tag="v2t")
            nc.vector.tensor_scalar_add(vg2t_sb, vg2t, 0.0)
            nc.gpsimd.dma_start(
                bass.AP(tensor=d_v2scr, offset=(BPC + b) * D,
                        ap=[[1, 128], [128, DC]]), vg2t_sb)
            vg2row = small.tile([1, D], bf16, tag="v2row")
            nc.gpsimd.dma_start(
                vg2row, bass.AP(tensor=d_v2scr, offset=(BPC + b) * D,
                                ap=[[D, 1], [1, D]]))
            return kgT, vg0, vg1, vg2row

        def g_attn(b, kgT, vg0, vg1, vg2row):
            # per-head scores as columns via block-diag-expanded q
            qblk = small.tile([128, DC, NH], bf16, tag="qblk")
            for d in range(DC):
                nc.vector.scalar_tensor_tensor(
                    out=qblk[:, d, :], in0=indh[:, d, :], scalar=QCLS[:, d, b:b + 1],
                    in1=biasblk[:, d, :], op0=ALU.mult, op1=ALU.add)
            scps = ps_mm.tile([128, 3 * NH], f32, tag="mm")
            nc.gpsimd.memset(scps, 0.0)
            for c in range(2):
                for d in range(DC):
                    MM(scps[:, NH * c:NH * (c + 1)],
                       lhsT=kgT[:, d, bass.ts(c, 128)], rhs=qblk[:, d, :],
                       start=False, stop=False, skip_group_check=True)
            for d in range(DC):
                MM(scps[0:1, 2 * NH:3 * NH], lhsT=kgT[:, d, 256:257], rhs=qblk[:, d, :],
                   start=False, stop=(d == DC - 1), skip_group_check=True)
            ecol = small.tile([128, 3 * NH], bf16, tag="ecol")
            nc.scalar.activation(ecol[:, 0:2 * NH], scps[:, 0:2 * NH], AF.Exp,
                                 scale=SCALE)
            nc.scalar.activation(ecol[0:1, 2 * NH:3 * NH], scps[0:1, 2 * NH:3 * NH],
                                 AF.Exp, scale=SCALE)
            zg = ps_b1.tile([1, NH], f32, tag="b1")
            MM(zg, lhsT=ones_col, rhs=ecol[:, 0:NH],
               start=True, stop=False, skip_group_check=True)
            MM(zg, lhsT=ones_col, rhs=ecol[:, NH:2 * NH],
               start=False, stop=False, skip_group_check=True)
            MM(zg, lhsT=ones_col[0:1, :], rhs=ecol[0:1, 2 * NH:3 * NH],
               start=False, stop=True, skip_group_check=True)
            rzgb = small.tile([1, NH], bf16, tag="rzgb")
            with nc.allow_low_precision(reason="softmax denom bf16 ok"):
                nc.vector.reciprocal(rzgb, zg)
            bzps = ps_mm.tile([128, NH], f32, tag="mm")
            MM(bzps, lhsT=ones_row, rhs=rzgb, start=True, stop=True)
            rzbc = small.tile([128, NH], bf16, tag="rzbc")
            nc.vector.tensor_scalar_add(rzbc, bzps, 0.0)
            ecoln = small.tile([128, 3 * NH], bf16, tag="ecoln")
            nc.vector.tensor_mul(ecoln[:, 0:NH], ecol[:, 0:NH], rzbc)
            nc.vector.tensor_mul(ecoln[:, NH:2 * NH], ecol[:, NH:2 * NH], rzbc)
            nc.vector.tensor_mul(ecoln[0:1, 2 * NH:3 * NH], ecol[0:1, 2 * NH:3 * NH],
                                 rzbc[0:1, :])
            agA = ps_mm.tile([1, 512], f32, tag="mm")
            agB = ps_mm.tile([1, 256], f32, tag="mm")
            nc.gpsimd.memset(agA, 0.0)
            nc.gpsimd.memset(agB, 0.0)
            for h in range(NH):
                dst = agA[0:1, h * HD:(h + 1) * HD] if h < 8 else \
                    agB[0:1, (h - 8) * HD:(h - 7) * HD]
                for c, vt in ((0, vg0), (1, vg1)):
                    MM(dst, lhsT=ecoln[:, NH * c + h:NH * c + h + 1],
                       rhs=vt[:, h * HD:(h + 1) * HD],
                       start=False, stop=False,
                       skip_group_check=True)
                MM(dst, lhsT=ecoln[0:1, 2 * NH + h:2 * NH + h + 1],
                   rhs=vg2row[0:1, h * HD:(h + 1) * HD],
                   start=False, stop=(h in (7, 11)), skip_group_check=True)
            agrow = small.tile([1, D], bf16, tag="agrow")
            nc.vector.tensor_scalar_add(agrow[:, 0:512], agA, 0.0)
            nc.vector.tensor_scalar_add(agrow[:, 512:768], agB, 0.0)
            nc.gpsimd.dma_start(AGROWS[b:b + 1, :], agrow)

        stage3 = {}
        stage3[0] = g_kv(0)
        stage3[1] = g_kv(1)
        for b in range(BPC):
            g_attn(b, *stage3.pop(b))
            if b + 2 < BPC:
                stage3[b + 2] = g_kv(b + 2)

        # ================= tail: wo_g, LN2, classifier =================
        attg = konst.tile([128, DC, BPC], bf16)
        for d in range(DC):
            tps = ps_mm.tile([128, BPC], bf16, tag="mm")
            nc.tensor.transpose(tps, AGROWS[:, bass.ts(d, 128)], ident[0:BPC, 0:BPC])
            evict(attg[:, d, :], tps)
        ogt = konst.tile([128, DC, BPC], bf16)
        for e in range(DC):
            ps = ps_mm.tile([128, BPC], f32, tag="mm")
            for f in range(DC):
                MM(ps, lhsT=wo_g[:, f, bass.ts(e, 128)], rhs=attg[:, f, :],
                   start=(f == 0), stop=(f == DC - 1))
            evict(ogt[:, e, :], ps, bias=bo_g_c[:, e:e + 1])
        # LN2
        st2 = ps_b1.tile([33, 512], f32, tag="b1")
        for d in range(DC):
            sq2 = small.tile([128, BPC], bf16, tag="sq2")
            nc.vector.tensor_mul(sq2, ogt[:, d, :], ogt[:, d, :])
            MM(st2[0:1, 0:BPC], lhsT=ones_col, rhs=ogt[:, d, :],
               start=(d == 0), stop=False, skip_group_check=True)
            MM(st2[32:33, 0:BPC], lhsT=ones_col, rhs=sq2,
               start=(d == 0), stop=(d == DC - 1), skip_group_check=True)
        mu2 = small.tile([1, BPC], bf16, tag="mu")
        nc.vector.tensor_scalar_mul(mu2, st2[0:1, 0:BPC], 1.0 / D)
        m22 = small.tile([1, BPC], f32, tag="m2")
        nc.vector.tensor_mul(m22, mu2, mu2)
        var2 = small.tile([1, BPC], f32, tag="varf")
        nc.vector.scalar_tensor_tensor(out=var2, in0=st2[32:33, 0:BPC], scalar=1.0 / D,
                                       in1=m22, op0=ALU.mult, op1=ALU.subtract)
        sd2 = small.tile([1, BPC], f32, tag="sd")
        nc.scalar.activation(sd2, var2, AF.Sqrt, bias=epsc)
        rstd2 = small.tile([1, BPC], bf16, tag="rstd")
        with nc.allow_low_precision(reason="rstd bf16 ok"):
            nc.vector.reciprocal(rstd2, sd2)
        bmu2 = ps_mm.tile([128, BPC], f32, tag="mm")
        MM(bmu2, lhsT=ones_row, rhs=mu2, start=True, stop=True)
        brs2 = ps_mm.tile([128, BPC], f32, tag="mm")
        MM(brs2, lhsT=ones_row, rhs=rstd2, start=True, stop=True)
        lng = konst.tile([128, DC, BPC], bf16)
        for d in range(DC):
            t1 = small.tile([128, BPC], bf16, tag="t1s")
            nc.vector.tensor_sub(t1, ogt[:, d, :], bmu2)
            t2 = small.tile([128, BPC], bf16, tag="t2s")
            nc.vector.tensor_mul(t2, t1, brs2)
            nc.scalar.activation(lng[:, d, :], t2, AF.Identity,
                                 bias=be2_c[:, d:d + 1], scale=g2_c[:, d:d + 1])
        # classifier
        outsb = konst.tile([BPC, NCLS], f32)
        for n0, nn in ((0, 512), (512, NCLS - 512)):
            ps = ps_mm.tile([BPC, 512], f32, tag="mm")
            for d in range(DC):
                MM(ps[:, :nn], lhsT=lng[:, d, :], rhs=wcls[:, d, n0:n0 + nn],
                   start=(d == 0), stop=(d == DC - 1))
            nc.vector.tensor_add(outsb[:, n0:n0 + nn], ps[:, :nn], bcls_r[:, n0:n0 + nn])
        nc.sync.dma_start(d_out.ap(), outsb)

    nc.compile()
    return nc


def prep_inputs(inputs):
    """numpy-only host prep: shard x; transpose/bcast/pack parameters."""
    f = lambda k: np.asarray(inputs[k], np.float32)
    x = f("x")
    pat = x[:, 0].reshape(B, GRID, PATCH, GRID, PATCH)
    pat = pat.transpose(0, 2, 4, 1, 3).reshape(B, PATCH * PATCH, NPATCH)
    patchesT = np.zeros((B, KP * KC, NPAD), np.float32)
    patchesT[:, :, 1:N] = pat
    patchesT = patchesT.reshape(B, KC, KP, NPAD).astype(BF16)

    wpT = f("w_patch").T.reshape(KC, KP, D).transpose(1, 0, 2).astype(BF16)

    pos = f("pos_embedding")[0]              # [257, 768]
    bp = np.zeros((D, NPAD), np.float32)
    bp[:, 1:N] = f("b_patch")[:, None] + pos[1:].T
    bp[:, 0] = f("cls_token")[0, 0] + pos[0]
    bp = bp.reshape(DC, 128, NPAD)

    indh = _indh()
    bq_g = f("bqkv_g")[:D].reshape(DC, 128, 1)
    biasblk = (indh * bq_g).astype(BF16)
    pairsel = np.zeros((2, 128), np.float32)
    pairsel[0, 0:64] = 1.0
    pairsel[1, 64:128] = 1.0

    shared = {
        "wpT": wpT,
        "bp": bp,
        "wqkvT_l": f("wqkv_l").T.reshape(DC, 128, E).astype(BF16),
        "woT_l": f("wo_l").T.reshape(DC, 128, D).astype(BF16),
        "wqkvT_g": f("wqkv_g").T.reshape(DC, 128, E).astype(BF16),
        "woT_g": f("wo_g").T.reshape(DC, 128, D).astype(BF16),
        "wclsT": f("w_cls").T.reshape(DC, 128, NCLS).astype(BF16),
        "band01": _band01(),
        "pairsel": pairsel.astype(BF16),
        "ident": np.eye(128, dtype=np.float32).astype(BF16),
        "indh": indh.astype(BF16),
        "biasblk": biasblk,
        "bqkv_l": f("bqkv_l"), "bo_l": f("bo_l"),
        "bqkv_g": f("bqkv_g"), "bo_g": f("bo_g"),
        "g1": f("g1"), "be1": f("be1"), "g2": f("g2"), "be2": f("be2"),
        "b_cls": f("b_cls"),
    }
    in_maps = []
    for c in range(NCORES):
        m = dict(shared)
        m["patchesT"] = patchesT[c * BPC:(c + 1) * BPC]
        in_maps.append(m)
    return in_maps


def kernel(**inputs) -> np.ndarray:
    if "nc" not in _CACHE:
        _CACHE["nc"] = build_nc(debug=False)
    nc = _CACHE["nc"]
    from concourse.bass_utils import run_bass_kernel_spmd
    in_maps = prep_inputs(inputs)
    res = run_bass_kernel_spmd(nc, in_maps, core_ids=list(range(NCORES)))
    return np.concatenate([r["logits"] for r in res.results], axis=0).astype(np.float32)


# revision 10
# speedup vs baseline: 1.3651x; 1.0100x over previous
"""Trainium2 Bass kernel for a 2-layer ViT (local banded MHA + global MHA, CLS head).

Contract: kernel(**inputs) takes the FULL fp32 inputs (as produced by
setup_inputs()) and returns the FULL [64, 1000] fp32 output. Internally the
batch (64) is sharded 8-ways across NeuronCores (data parallel); parameters are
replicated. Self-contained: shapes/sharding hardcoded.

v2 design notes (vs the first working version):
 - activations transposed on chip: [D=768 (6 x 128 partitions), Ntok], tokens
   padded 257 -> 264 columns.
 - local banded attention (radius 1) per head: three k-major score grids land
   in one PSUM bank [128, 269]; one Exp (1/sqrt(hd) scale fused) evicts to
   bf16, one 0/1 band-mask multiply on DVE kills off-band entries. AV uses
   disjoint q-ranges per chunk plus 1-column boundary matmuls (contraction-64
   at base partition 64 where needed, relying on the mask zeros), so no PSUM
   zero-init matmuls and no -inf mask tensor.
 - softmax denominators: per-pair row sums at partitions {0,32} of one PSUM
   bank; one strided DVE reciprocal; a rank-1 "pairsel" matmul broadcasts the
   reciprocals across partitions; the divide fuses into the PSUM->SBUF
   eviction multiply.
 - the 257th token's V is computed as columns (36 1-col matmuls) and bounced
   through DRAM into row layout, avoiding full-width 768-col matmuls for an
   8-token chunk.
 - evictions split between Act and DVE; patch-embed bias adds and LN squares
   run on the idle GPSIMD (Pool) engine.
 - per-item work is software-pipelined (embed/qkv of item b+1 enqueued before
   attention of item b) so the tensor engine rarely waits on cross-engine
   softmax round trips.
 - layer 2 computes K/V for all tokens but Q/attention/output only for the
   CLS token; QCLS accumulates incrementally per item (no global barrier).
"""

import numpy as np
import ml_dtypes
from contextlib import ExitStack

BF16 = ml_dtypes.bfloat16

B, NCORES, BPC = 64, 8, 8
IMAGE, PATCH, GRID = 224, 14, 16
NPATCH, N, NPAD = 256, 257, 264
D, NH, HD, E, NCLS = 768, 12, 64, 2304, 1000
DC = D // 128             # 6 d-chunks
KP, KC = 98, 2            # patch-pixel contraction chunks: 196 = 2*98
SCALE = 1.0 / np.sqrt(HD)
SCW = 269                 # score grid cols: 130 (g0) + 130 (g1) + 9 (g2)

_CACHE = {}


def _indh():
    ind = np.zeros((DC, 128, NH), np.float32)
    for dc in range(DC):
        for p in range(128):
            ind[dc, p, (128 * dc + p) // HD] = 1.0
    return ind


def _band01():
    m = np.zeros((128, SCW), np.float32)
    for kl in range(128):
        for j in range(130):
            # g0: k = kl, q = j (j <= 128 real)
            if j <= 128 and abs(kl - j) <= 1:
                m[kl, j] = 1.0
            # g1: k = 128 + kl, q = 127 + j
            if abs(kl + 1 - j) <= 1:
                m[kl, 130 + j] = 1.0
    # g2: row 0 = k 256; cols j: q = 255 + j (boundary, q256, pads)
    for j in range(9):
        m[0, 260 + j] = 1.0
    return m.astype(BF16)


def build_nc(debug=False):
    import concourse.bacc as bacc
    import concourse.tile as tile
    from concourse import mybir
    import concourse.bass as bass

    f32, bf16 = mybir.dt.float32, mybir.dt.bfloat16
    AF, ALU = mybir.ActivationFunctionType, mybir.AluOpType

    nc = bacc.Bacc("TRN2", target_bir_lowering=False, debug=False)

    # ---- DRAM I/O ----
    d_pt = nc.dram_tensor("patchesT", [BPC, KC, KP, NPAD], bf16, kind="ExternalInput")
    d_wpT = nc.dram_tensor("wpT", [KP, KC, D], bf16, kind="ExternalInput")
    d_bp = nc.dram_tensor("bp", [DC, 128, NPAD], f32, kind="ExternalInput")
    d_wqkvT_l = nc.dram_tensor("wqkvT_l", [DC, 128, E], bf16, kind="ExternalInput")
    d_woT_l = nc.dram_tensor("woT_l", [DC, 128, D], bf16, kind="ExternalInput")
    d_wqkvT_g = nc.dram_tensor("wqkvT_g", [DC, 128, E], bf16, kind="ExternalInput")
    d_woT_g = nc.dram_tensor("woT_g", [DC, 128, D], bf16, kind="ExternalInput")
    d_wclsT = nc.dram_tensor("wclsT", [DC, 128, NCLS], bf16, kind="ExternalInput")
    d_band = nc.dram_tensor("band01", [128, SCW], bf16, kind="ExternalInput")
    d_psel = nc.dram_tensor("pairsel", [2, 128], bf16, kind="ExternalInput")
    d_ident = nc.dram_tensor("ident", [128, 128], bf16, kind="ExternalInput")
    d_indh = nc.dram_tensor("indh", [DC, 128, NH], bf16, kind="ExternalInput")
    d_biasblk = nc.dram_tensor("biasblk", [DC, 128, NH], bf16, kind="ExternalInput")
    d_bqkv_l = nc.dram_tensor("bqkv_l", [E], f32, kind="ExternalInput")
    d_bo_l = nc.dram_tensor("bo_l", [D], f32, kind="ExternalInput")
    d_bqkv_g = nc.dram_tensor("bqkv_g", [E], f32, kind="ExternalInput")
    d_bo_g = nc.dram_tensor("bo_g", [D], f32, kind="ExternalInput")
    d_g1 = nc.dram_tensor("g1", [D], f32, kind="ExternalInput")
    d_be1 = nc.dram_tensor("be1", [D], f32, kind="ExternalInput")
    d_g2 = nc.dram_tensor("g2", [D], f32, kind="ExternalInput")
    d_be2 = nc.dram_tensor("be2", [D], f32, kind="ExternalInput")
    d_bcls = nc.dram_tensor("b_cls", [NCLS], f32, kind="ExternalInput")
    d_v2scr = nc.dram_tensor("v2scr", [2, BPC, D], bf16, kind="Internal")
    d_out = nc.dram_tensor("logits", [BPC, NCLS], f32, kind="ExternalOutput")

    with tile.TileContext(nc) as tc, ExitStack() as ctx:
        konst = ctx.enter_context(tc.tile_pool(name="konst", bufs=1))
        acts = ctx.enter_context(tc.tile_pool(name="acts", bufs=2))
        small = ctx.enter_context(tc.tile_pool(name="small", bufs=2))
        # PSUM: 8 banks total: mm 2 + sc 2 + av(+zh) 2 + b1 2
        ps_mm = ctx.enter_context(tc.tile_pool(name="ps_mm", bufs=2, space="PSUM"))
        ps_sc = ctx.enter_context(tc.tile_pool(name="ps_sc", bufs=2, space="PSUM"))
        ps_av = ctx.enter_context(tc.tile_pool(name="ps_av", bufs=2, space="PSUM"))
        ps_b1 = ctx.enter_context(tc.tile_pool(name="ps_b1", bufs=2, space="PSUM"))

        # ---- persistent SBUF ----
        wpT = konst.tile([KP, KC, D], bf16)
        nc.sync.dma_start(wpT, d_wpT.ap())
        wqkv_l = konst.tile([128, DC, E], bf16)
        wo_l = konst.tile([128, DC, D], bf16)
        wqkv_g = konst.tile([128, DC, E], bf16)
        wo_g = konst.tile([128, DC, D], bf16)
        wcls = konst.tile([128, DC, NCLS], bf16)
        bp = konst.tile([128, DC, NPAD], f32)
        for d in range(DC):
            nc.sync.dma_start(bp[:, d, :], d_bp.ap()[d])
            nc.sync.dma_start(wqkv_l[:, d, :], d_wqkvT_l.ap()[d])
            nc.scalar.dma_start(wo_l[:, d, :], d_woT_l.ap()[d])
        for d in range(DC):
            nc.scalar.dma_start(wqkv_g[:, d, :], d_wqkvT_g.ap()[d])
            nc.scalar.dma_start(wo_g[:, d, :], d_woT_g.ap()[d])
            nc.scalar.dma_start(wcls[:, d, :], d_wclsT.ap()[d])
        band01 = konst.tile([128, SCW], bf16)
        nc.sync.dma_start(band01, d_band.ap())
        pairsel = konst.tile([2, 128], bf16)
        nc.sync.dma_start(pairsel, d_psel.ap())
        ident = konst.tile([128, 128], bf16)
        nc.sync.dma_start(ident, d_ident.ap())
        indh = konst.tile([128, DC, NH], bf16)
        biasblk = konst.tile([128, DC, NH], bf16)
        for d in range(DC):
            nc.sync.dma_start(indh[:, d, :], d_indh.ap()[d])
            nc.sync.dma_start(biasblk[:, d, :], d_biasblk.ap()[d])
        bqkv_l_c = konst.tile([128, 18], f32)
        nc.sync.dma_start(bqkv_l_c, d_bqkv_l.ap().rearrange("(j p) -> p j", p=128))
        bqkv_g_c = konst.tile([128, 18], f32)
        nc.sync.dma_start(bqkv_g_c, d_bqkv_g.ap().rearrange("(j p) -> p j", p=128))
        bo_l_c = konst.tile([128, DC], f32)
        nc.sync.dma_start(bo_l_c, d_bo_l.ap().rearrange("(j p) -> p j", p=128))
        bo_g_c = konst.tile([128, DC], f32)
        nc.sync.dma_start(bo_g_c, d_bo_g.ap().rearrange("(j p) -> p j", p=128))
        g1_c = konst.tile([128, DC], f32)
        nc.sync.dma_start(g1_c, d_g1.ap().rearrange("(j p) -> p j", p=128))
        be1_c = konst.tile([128, DC], f32)
        nc.sync.dma_start(be1_c, d_be1.ap().rearrange("(j p) -> p j", p=128))
        g2_c = konst.tile([128, DC], f32)
        nc.sync.dma_start(g2_c, d_g2.ap().rearrange("(j p) -> p j", p=128))
        be2_c = konst.tile([128, DC], f32)
        nc.sync.dma_start(be2_c, d_be2.ap().rearrange("(j p) -> p j", p=128))
        bcls_r = konst.tile([BPC, NCLS], f32)
        nc.sync.dma_start(
            bcls_r, bass.AP(tensor=d_bcls, offset=0, ap=[[0, BPC], [1, NCLS]])
        )
        ones_col = konst.tile([128, 1], bf16)
        nc.vector.memset(ones_col, 1.0)
        ones_row = konst.tile([1, 128], bf16)
        nc.vector.memset(ones_row, 1.0)
        epsc = konst.tile([1, 1], f32)
        nc.vector.memset(epsc, 1e-5)
        zrow = konst.tile([1, 512], bf16)
        nc.vector.memset(zrow, 0.0)

        LOCAL = konst.tile([128, BPC, DC, NPAD], bf16)   # post-LN1, all batches
        AGROWS = konst.tile([BPC, D], bf16)              # global attn out rows
        QCLS = konst.tile([128, DC, BPC], f32)           # global q for CLS (no bias)

        def evict(dst, src, bias=None, scale=1.0):
            if bias is None:
                nc.scalar.activation(dst, src, AF.Copy, scale=scale)
            else:
                nc.scalar.activation(dst, src, AF.Identity, bias=bias, scale=scale)

        MM = nc.tensor.matmul

        # --------------- pass 1 stages ---------------
        def s_embed(b):
            """patch embed + pos bias -> tokT(b)"""
            pt = acts.tile([KP, KC, NPAD], bf16, tag="pt")
            for k in range(KC):
                nc.sync.dma_start(pt[:, k, :], d_pt.ap()[b, k])
            tokT = acts.tile([128, DC, NPAD], bf16, tag="tokT")
            for d in range(DC):
                ps = ps_mm.tile([128, NPAD], f32, tag="mm")
                for k in range(KC):
                    MM(ps, lhsT=wpT[:, k, bass.ts(d, 128)], rhs=pt[:, k, :],
                       start=(k == 0), stop=(k == KC - 1))
                nc.gpsimd.tensor_add(tokT[:, d, :], ps, bp[:, d, :])
            return tokT

        def s_qkv(b, tokT, wqkv, bias_c, layer):
            """Q^T,K^T e-chunks (layer 1 only); V token-major chunks 0/1;
            tail-token V row via DRAM bounce. Returns (qkT, v0, v1, v2row)."""
            qkT = None
            if layer == 0:
                qkT = acts.tile([128, 12, NPAD], bf16, tag="qkT")
                for e in range(12):
                    ps = ps_mm.tile([128, NPAD], f32, tag="mm")
                    for d in range(DC):
                        MM(ps, lhsT=wqkv[:, d, bass.ts(e, 128)], rhs=tokT[:, d, :],
                           start=(d == 0), stop=(d == DC - 1))
                    if e % 2 == 0:
                        evict(qkT[:, e, :], ps, bias=bias_c[:, e:e + 1])
                    else:
                        nc.vector.tensor_scalar_add(qkT[:, e, :], ps,
                                                    bias_c[:, e:e + 1])
            v0 = acts.tile([128, D], bf16, tag="v0")
            v1 = acts.tile([128, D], bf16, tag="v1")
            for t, vt in ((0, v0), (1, v1)):
                for n0, nn in ((0, 512), (512, 256)):
                    ps = ps_mm.tile([128, nn], f32, tag="mm")
                    for d in range(DC):
                        MM(ps, lhsT=tokT[:, d, bass.ts(t, 128)],
                           rhs=wqkv[:, d, 2 * D + n0:2 * D + n0 + nn],
                           start=(d == 0), stop=(d == DC - 1))
                    evict(vt[:, n0:n0 + nn], ps)
            # tail token (256): V as columns -> DRAM bounce -> row layout
            v2t = ps_b1.tile([128, DC], f32, tag="b1")
            MM(v2t, lhsT=ones_row, rhs=zrow[0:1, 0:DC],
               start=True, stop=False, skip_group_check=True)
            for c in range(DC):
                for d in range(DC):
                    MM(v2t[:, c:c + 1],
                       lhsT=wqkv[:, d, 2 * D + 128 * c:2 * D + 128 * (c + 1)],
                       rhs=tokT[:, d, 256:257],
                       start=False, stop=(c == DC - 1 and d == DC - 1),
                       skip_group_check=True)
            v2t_sb = small.tile([128, DC], bf16, tag="v2t")
            nc.vector.tensor_scalar_add(v2t_sb, v2t, 0.0)
            nc.sync.dma_start(
                bass.AP(tensor=d_v2scr, offset=(layer * BPC + b) * D,
                        ap=[[1, 128], [128, DC]]), v2t_sb)
            v2row = small.tile([1, D], bf16, tag="v2row")
            nc.sync.dma_start(
                v2row, bass.AP(tensor=d_v2scr, offset=(layer * BPC + b) * D,
                               ap=[[D, 1], [1, D]]))
            return qkT, v0, v1, v2row

        def s_attn(b, qkT, v0, v1, v2row):
            """local banded attention -> AVT(b) (normalized, bf16)."""
            et = acts.tile([128, NH, SCW], bf16, tag="et", bufs=1)
            AVT = acts.tile([128, DC, NPAD], bf16, tag="AVT")

            def sps_head(h):
                qh = qkT[64 * (h % 2):64 * (h % 2) + 64, h // 2, :]
                kh = qkT[64 * (h % 2):64 * (h % 2) + 64, 6 + h // 2, :]
                sc = ps_sc.tile([128, SCW], f32, tag="sc")
                MM(sc[:, 0:130], lhsT=kh[:, 0:128], rhs=qh[:, 0:130],
                   start=True, stop=False, skip_group_check=True)
                MM(sc[:, 130:260], lhsT=kh[:, 128:256], rhs=qh[:, 127:257],
                   start=True, stop=False, skip_group_check=True)
                MM(sc[0:1, 260:269], lhsT=kh[:, 256:257], rhs=qh[:, 255:264],
                   start=True, stop=True, skip_group_check=True)
                nc.vector.memset(sc[1:128, 260:269], 0.0)
                nc.scalar.activation(et[:, h, :], sc, AF.Exp, scale=SCALE)
                nc.vector.tensor_mul(et[:, h, :], et[:, h, :], band01)

            def z_head(zh, h, hh, last):
                r = 32 * hh
                MM(zh[r:r + 1, 0:128], lhsT=ones_col, rhs=et[:, h, 0:128],
                   start=False, stop=False, skip_group_check=True)
                MM(zh[r:r + 1, 128:256], lhsT=ones_col, rhs=et[:, h, 131:259],
                   start=False, stop=False, skip_group_check=True)
                MM(zh[r:r + 1, 256:264], lhsT=ones_col[0:1, :],
                   rhs=et[0:1, h, 261:269],
                   start=False, stop=False, skip_group_check=True)
                # boundary columns (contraction-32 reads rely on mask zeros)
                MM(zh[r:r + 1, 128:129], lhsT=ones_col[64:128, :],
                   rhs=et[64:128, h, 128:129],
                   start=False, stop=False, skip_group_check=True)
                MM(zh[r:r + 1, 127:128], lhsT=ones_col[0:1, :],
                   rhs=et[0:1, h, 130:131],
                   start=False, stop=False, skip_group_check=True)
                MM(zh[r:r + 1, 256:257], lhsT=ones_col[64:128, :],
                   rhs=et[64:128, h, 259:260],
                   start=False, stop=False, skip_group_check=True)
                MM(zh[r:r + 1, 255:256], lhsT=ones_col[0:1, :],
                   rhs=et[0:1, h, 260:261],
                   start=False, stop=last, skip_group_check=True)

            def av_head(av, h, hh, last):
                r0 = 64 * hh
                hs = slice(h * HD, (h + 1) * HD)
                v2r = v2row[0:1, hs]
                MM(av[r0:r0 + 64, 0:128], lhsT=v0[:, hs], rhs=et[:, h, 0:128],
                   start=False, stop=False, skip_group_check=True)
                MM(av[r0:r0 + 64, 128:256], lhsT=v1[:, hs], rhs=et[:, h, 131:259],
                   start=False, stop=False, skip_group_check=True)
                MM(av[r0:r0 + 64, 256:264], lhsT=v2r, rhs=et[0:1, h, 261:269],
                   start=False, stop=False, skip_group_check=True)
                MM(av[r0:r0 + 64, 128:129], lhsT=v0[64:128, hs],
                   rhs=et[64:128, h, 128:129],
                   start=False, stop=False, skip_group_check=True)
                MM(av[r0:r0 + 64, 127:128], lhsT=v1[0:1, hs],
                   rhs=et[0:1, h, 130:131],
                   start=False, stop=False, skip_group_check=True)
                MM(av[r0:r0 + 64, 256:257], lhsT=v1[64:128, hs],
                   rhs=et[64:128, h, 259:260],
                   start=False, stop=False, skip_group_check=True)
                MM(av[r0:r0 + 64, 255:256], lhsT=v2r, rhs=et[0:1, h, 260:261],
                   start=False, stop=last, skip_group_check=True)

            sps_head(0)
            sps_head(1)
            state = {}
            for dc in range(DC):
                zh = ps_av.tile([33, 512], f32, tag="av")
                MM(zh[0:33, 0:NPAD], lhsT=ones_row[0:1, 0:33], rhs=zrow[0:1, 0:NPAD],
                   start=True, stop=False, skip_group_check=True)
                z_head(zh, 2 * dc, 0, False)
                z_head(zh, 2 * dc + 1, 1, True)
                rz2 = small.tile([2, NPAD], bf16, tag="rz2")
                with nc.allow_low_precision(reason="softmax denom, bf16 ok"):
                    nc.vector.reciprocal(rz2, zh[0:33:32, 0:NPAD])
                if dc < DC - 1:
                    sps_head(2 * dc + 2)
                    sps_head(2 * dc + 3)
                av = ps_av.tile([128, NPAD], f32, tag="av")
                MM(av, lhsT=ones_row, rhs=zrow[0:1, 0:NPAD],
                   start=True, stop=False, skip_group_check=True)
                av_head(av, 2 * dc, 0, False)
                av_head(av, 2 * dc + 1, 1, True)
                nps = ps_b1.tile([128, NPAD], f32, tag="b1")
                MM(nps, lhsT=pairsel, rhs=rz2, start=True, stop=True)
                nc.vector.tensor_mul(AVT[:, dc, :], av, nps)
            return AVT

        def s_out_ln(b, AVT):
            """out-proj + LayerNorm1 -> LOCAL[:, b]; QCLS contribution."""
            x1 = acts.tile([128, DC, NPAD], bf16, tag="x1")
            for e in range(DC):
                ps = ps_mm.tile([128, NPAD], f32, tag="mm")
                for f in range(DC):
                    MM(ps, lhsT=wo_l[:, f, bass.ts(e, 128)], rhs=AVT[:, f, :],
                       start=(f == 0), stop=(f == DC - 1))
                if e % 2 == 0:
                    evict(x1[:, e, :], ps, bias=bo_l_c[:, e:e + 1])
                else:
                    nc.vector.tensor_scalar_add(x1[:, e, :], ps, bo_l_c[:, e:e + 1])
            # stats: rows 0 (sum x) and 32 (sum x^2) of one PSUM tile
            st = ps_b1.tile([33, 512], f32, tag="b1")
            sq = acts.tile([128, NPAD], bf16, tag="sq")
            for d in range(DC):
                nc.gpsimd.tensor_mul(sq, x1[:, d, :], x1[:, d, :])
                MM(st[0:1, 0:NPAD], lhsT=ones_col, rhs=x1[:, d, :],
                   start=(d == 0), stop=False, skip_group_check=True)
                MM(st[32:33, 0:NPAD], lhsT=ones_col, rhs=sq,
                   start=(d == 0), stop=(d == DC - 1), skip_group_check=True)
            murow = small.tile([1, NPAD], bf16, tag="mu")
            nc.vector.tensor_scalar_mul(murow, st[0:1, 0:NPAD], 1.0 / D)
            m2 = small.tile([1, NPAD], f32, tag="m2")
            nc.vector.tensor_mul(m2, murow, murow)
            varf = small.tile([1, NPAD], f32, tag="varf")
            nc.vector.scalar_tensor_tensor(out=varf, in0=st[32:33, 0:NPAD], scalar=1.0 / D,
                                           in1=m2, op0=ALU.mult, op1=ALU.subtract)
            lnv = small.tile([1, NPAD], f32, tag="sd")
            nc.scalar.activation(lnv, varf, AF.Ln, bias=epsc)
            rstd = small.tile([1, NPAD], bf16, tag="rstd")
            nc.scalar.activation(rstd, lnv, AF.Exp, scale=-0.5)
            bmu_ps = ps_mm.tile([128, NPAD], f32, tag="mm")
            MM(bmu_ps, lhsT=ones_row, rhs=murow, start=True, stop=True)
            bmu = acts.tile([128, NPAD], bf16, tag="bmu")
            evict(bmu, bmu_ps)
            brs_ps = ps_mm.tile([128, NPAD], f32, tag="mm")
            MM(brs_ps, lhsT=ones_row, rhs=rstd, start=True, stop=True)
            brs = acts.tile([128, NPAD], bf16, tag="brs")
            evict(brs, brs_ps)
            for d in range(DC):
                t1 = acts.tile([128, NPAD], bf16, tag="t1")
                nc.vector.tensor_sub(t1, x1[:, d, :], bmu)
                t2 = acts.tile([128, NPAD], bf16, tag="t2")
                nc.vector.tensor_mul(t2, t1, brs)
                nc.scalar.activation(LOCAL[:, b, d, :], t2, AF.Identity,
                                     bias=be1_c[:, d:d + 1], scale=g1_c[:, d:d + 1])
            # QCLS contribution for this item (global q for CLS, bias-free)
            qc = ps_b1.tile([128, DC], f32, tag="b1")
            MM(qc, lhsT=ones_row, rhs=zrow[0:1, 0:DC],
               start=True, stop=False, skip_group_check=True)
            for e in range(DC):
                for d in range(DC):
                    MM(qc[:, e:e + 1], lhsT=wqkv_g[:, d, bass.ts(e, 128)],
                       rhs=LOCAL[:, b, d, 0:1],
                       start=False, stop=(e == DC - 1 and d == DC - 1),
                       skip_group_check=True)
            nc.vector.tensor_scalar_add(QCLS[:, :, b], qc, 0.0)

        # --------------- pass 1: software-pipelined emission ---------------
        stage1 = {}
        def emit_front(b):
            tokT = s_embed(b)
            stage1[b] = s_qkv(b, tokT, wqkv_l, bqkv_l_c, 0)
        emit_front(0)
        emit_front(1)
        for b in range(BPC):
            AVT = s_attn(b, *stage1.pop(b))
            if b + 2 < BPC:
                emit_front(b + 2)
            s_out_ln(b, AVT)

        # --------------- pass 3 stages (global attention) ---------------
        def g_kv(b):
            kgT = acts.tile([128, DC, NPAD], bf16, tag="kgT")
            for e in range(DC):
                ps = ps_mm.tile([128, NPAD], f32, tag="mm")
                for d in range(DC):
                    MM(ps, lhsT=wqkv_g[:, d, D + 128 * e:D + 128 * (e + 1)],
                       rhs=LOCAL[:, b, d, :], start=(d == 0), stop=(d == DC - 1))
                if e % 2 == 0:
                    evict(kgT[:, e, :], ps, bias=bqkv_g_c[:, 6 + e:7 + e])
                else:
                    nc.vector.tensor_scalar_add(kgT[:, e, :], ps,
                                                bqkv_g_c[:, 6 + e:7 + e])
            vg0 = acts.tile([128, D], bf16, tag="v0")
            vg1 = acts.tile([128, D], bf16, tag="v1")
            for t, vt in ((0, vg0), (1, vg1)):
                for n0, nn in ((0, 512), (512, 256)):
                    ps = ps_mm.tile([128, nn], f32, tag="mm")
                    for d in range(DC):
                        MM(ps, lhsT=LOCAL[:, b, d, bass.ts(t, 128)],
                           rhs=wqkv_g[:, d, 2 * D + n0:2 * D + n0 + nn],
                           start=(d == 0), stop=(d == DC - 1))
                    evict(vt[:, n0:n0 + nn], ps)
            vg2t = ps_b1.tile([128, DC], f32, tag="b1")
            MM(vg2t, lhsT=ones_row, rhs=zrow[0:1, 0:DC],
               start=True, stop=False, skip_group_check=True)
            for c in range(DC):
                for d in range(DC):
                    MM(vg2t[:, c:c + 1],
                       lhsT=wqkv_g[:, d, 2 * D + 128 * c:2 * D + 128 * (c + 1)],
                       rhs=LOCAL[:, b, d, 256:257],
                       start=False, stop=(c == DC - 1 and d == DC - 1),
                       skip_group_check=True)
            vg2t_sb = small.tile([128, DC], bf16, tag="v2t")
            nc.vector.tensor_scalar_add(vg2t_sb, vg2t, 0.0)
            nc.sync.dma_start(
                bass.AP(tensor=d_v2scr, offset=(BPC + b) * D,
                        ap=[[1, 128], [128, DC]]), vg2t_sb)
            vg2row = small.tile([1, D], bf16, tag="v2row")
            nc.sync.dma_start(
                vg2row, bass.AP(tensor=d_v2scr, offset=(BPC + b) * D,
                                ap=[[D, 1], [1, D]]))
            return kgT, vg0, vg1, vg2row

        def g_attn(b, kgT, vg0, vg1, vg2row):
            # per-head scores as columns via block-diag-expanded q
            qblk = small.tile([128, DC, NH], bf16, tag="qblk")
            for d in range(DC):
                nc.vector.scalar_tensor_tensor(
                    out=qblk[:, d, :], in0=indh[:, d, :], scalar=QCLS[:, d, b:b + 1],
                    in1=biasblk[:, d, :], op0=ALU.mult, op1=ALU.add)
            scps = ps_mm.tile([128, 3 * NH], f32, tag="mm")
            MM(scps, lhsT=ones_row, rhs=zrow[0:1, 0:3 * NH],
               start=True, stop=False, skip_group_check=True)
            for c in range(2):
                for d in range(DC):
                    MM(scps[:, NH * c:NH * (c + 1)],
                       lhsT=kgT[:, d, bass.ts(c, 128)], rhs=qblk[:, d, :],
                       start=False, stop=False, skip_group_check=True)
            for d in range(DC):
                MM(scps[0:1, 2 * NH:3 * NH], lhsT=kgT[:, d, 256:257], rhs=qblk[:, d, :],
                   start=False, stop=(d == DC - 1), skip_group_check=True)
            ecol = small.tile([128, 3 * NH], bf16, tag="ecol")
            nc.scalar.activation(ecol[:, 0:2 * NH], scps[:, 0:2 * NH], AF.Exp,
                                 scale=SCALE)
            nc.scalar.activation(ecol[0:1, 2 * NH:3 * NH], scps[0:1, 2 * NH:3 * NH],
                                 AF.Exp, scale=SCALE)
            zg = ps_b1.tile([1, NH], f32, tag="b1")
            MM(zg, lhsT=ones_col, rhs=ecol[:, 0:NH],
               start=True, stop=False, skip_group_check=True)
            MM(zg, lhsT=ones_col, rhs=ecol[:, NH:2 * NH],
               start=False, stop=False, skip_group_check=True)
            MM(zg, lhsT=ones_col[0:1, :], rhs=ecol[0:1, 2 * NH:3 * NH],
               start=False, stop=True, skip_group_check=True)
            rzgb = small.tile([1, NH], bf16, tag="rzgb")
            with nc.allow_low_precision(reason="softmax denom bf16 ok"):
                nc.vector.reciprocal(rzgb, zg)
            bzps = ps_mm.tile([128, NH], f32, tag="mm")
            MM(bzps, lhsT=ones_row, rhs=rzgb, start=True, stop=True)
            rzbc = small.tile([128, NH], bf16, tag="rzbc")
            nc.vector.tensor_scalar_add(rzbc, bzps, 0.0)
            ecoln = small.tile([128, 3 * NH], bf16, tag="ecoln")
            nc.vector.tensor_mul(ecoln[:, 0:NH], ecol[:, 0:NH], rzbc)
            nc.vector.tensor_mul(ecoln[:, NH:2 * NH], ecol[:, NH:2 * NH], rzbc)
            nc.vector.tensor_mul(ecoln[0:1, 2 * NH:3 * NH], ecol[0:1, 2 * NH:3 * NH],
                                 rzbc[0:1, :])
            agA = ps_mm.tile([1, 512], f32, tag="mm")
            agB = ps_mm.tile([1, 256], f32, tag="mm")
            MM(agA, lhsT=ones_row[0:1, 0:1], rhs=zrow[0:1, 0:512],
               start=True, stop=False, skip_group_check=True)
            MM(agB, lhsT=ones_row[0:1, 0:1], rhs=zrow[0:1, 0:256],
               start=True, stop=False, skip_group_check=True)
            for h in range(NH):
                dst = agA[0:1, h * HD:(h + 1) * HD] if h < 8 else \
                    agB[0:1, (h - 8) * HD:(h - 7) * HD]
                for c, vt in ((0, vg0), (1, vg1)):
                    MM(dst, lhsT=ecoln[:, NH * c + h:NH * c + h + 1],
                       rhs=vt[:, h * HD:(h + 1) * HD],
                       start=False, stop=False,
                       skip_group_check=True)
                MM(dst, lhsT=ecoln[0:1, 2 * NH + h:2 * NH + h + 1],
                   rhs=vg2row[0:1, h * HD:(h + 1) * HD],
                   start=False, stop=(h in (7, 11)), skip_group_check=True)
            agrow = small.tile([1, D], bf16, tag="agrow")
            nc.vector.tensor_scalar_add(agrow[:, 0:512], agA, 0.0)
            nc.vector.tensor_scalar_add(agrow[:, 512:768], agB, 0.0)
            nc.sync.dma_start(AGROWS[b:b + 1, :], agrow)

        stage3 = {}
        stage3[0] = g_kv(0)
        stage3[1] = g_kv(1)
        for b in range(BPC):
            g_attn(b, *stage3.pop(b))
            if b + 2 < BPC:
                stage3[b + 2] = g_kv(b + 2)

        # ================= tail: wo_g, LN2, classifier =================
        attg = konst.tile([128, DC, BPC], bf16)
        for d in range(DC):
            tps = ps_mm.tile([128, BPC], bf16, tag="mm")
            nc.tensor.transpose(tps, AGROWS[:, bass.ts(d, 128)], ident[0:BPC, 0:BPC])
            evict(attg[:, d, :], tps)
        ogt = konst.tile([128, DC, BPC], bf16)
        for e in range(DC):
            ps = ps_mm.tile([128, BPC], f32, tag="mm")
            for f in range(DC):
                MM(ps, lhsT=wo_g[:, f, bass.ts(e, 128)], rhs=attg[:, f, :],
                   start=(f == 0), stop=(f == DC - 1))
            evict(ogt[:, e, :], ps, bias=bo_g_c[:, e:e + 1])
        # LN2
        st2 = ps_b1.tile([33, 512], f32, tag="b1")
        for d in range(DC):
            sq2 = small.tile([128, BPC], bf16, tag="sq2")
            nc.vector.tensor_mul(sq2, ogt[:, d, :], ogt[:, d, :])
            MM(st2[0:1, 0:BPC], lhsT=ones_col, rhs=ogt[:, d, :],
               start=(d == 0), stop=False, skip_group_check=True)
            MM(st2[32:33, 0:BPC], lhsT=ones_col, rhs=sq2,
               start=(d == 0), stop=(d == DC - 1), skip_group_check=True)
        mu2 = small.tile([1, BPC], bf16, tag="mu")
        nc.vector.tensor_scalar_mul(mu2, st2[0:1, 0:BPC], 1.0 / D)
        m22 = small.tile([1, BPC], f32, tag="m2")
        nc.vector.tensor_mul(m22, mu2, mu2)
        var2 = small.tile([1, BPC], f32, tag="varf")
        nc.vector.scalar_tensor_tensor(out=var2, in0=st2[32:33, 0:BPC], scalar=1.0 / D,
                                       in1=m22, op0=ALU.mult, op1=ALU.subtract)
        lnv2 = small.tile([1, BPC], f32, tag="sd")
        nc.scalar.activation(lnv2, var2, AF.Ln, bias=epsc)
        rstd2 = small.tile([1, BPC], bf16, tag="rstd")
        nc.scalar.activation(rstd2, lnv2, AF.Exp, scale=-0.5)
        bmu2 = ps_mm.tile([128, BPC], f32, tag="mm")
        MM(bmu2, lhsT=ones_row, rhs=mu2, start=True, stop=True)
        brs2 = ps_mm.tile([128, BPC], f32, tag="mm")
        MM(brs2, lhsT=ones_row, rhs=rstd2, start=True, stop=True)
        lng = konst.tile([128, DC, BPC], bf16)
        for d in range(DC):
            t1 = small.tile([128, BPC], bf16, tag="t1s")
            nc.vector.tensor_sub(t1, ogt[:, d, :], bmu2)
            t2 = small.tile([128, BPC], bf16, tag="t2s")
            nc.vector.tensor_mul(t2, t1, brs2)
            nc.scalar.activation(lng[:, d, :], t2, AF.Identity,
                                 bias=be2_c[:, d:d + 1], scale=g2_c[:, d:d + 1])
        # classifier
        outsb = konst.tile([BPC, NCLS], f32)
        for n0, nn in ((0, 512), (512, NCLS - 512)):
            ps = ps_mm.tile([BPC, 512], f32, tag="mm")
            for d in range(DC):
                MM(ps[:, :nn], lhsT=lng[:, d, :], rhs=wcls[:, d, n0:n0 + nn],
                   start=(d == 0), stop=(d == DC - 1))
            nc.vector.tensor_add(outsb[:, n0:n0 + nn], ps[:, :nn], bcls_r[:, n0:n0 + nn])
        nc.sync.dma_start(d_out.ap(), outsb)

    nc.compile()
    return nc


def prep_inputs(inputs):
    """numpy-only host prep: shard x; transpose/bcast/pack parameters."""
    f = lambda k: np.asarray(inputs[k], np.float32)
    x = f("x")
    pat = x[:, 0].reshape(B, GRID, PATCH, GRID, PATCH)
    pat = pat.transpose(0, 2, 4, 1, 3).reshape(B, PATCH * PATCH, NPATCH)
    patchesT = np.zeros((B, KP * KC, NPAD), np.float32)
    patchesT[:, :, 1:N] = pat
    patchesT = patchesT.reshape(B, KC, KP, NPAD).astype(BF16)

    wpT = f("w_patch").T.reshape(KC, KP, D).transpose(1, 0, 2).astype(BF16)

    pos = f("pos_embedding")[0]              # [257, 768]
    bp = np.zeros((D, NPAD), np.float32)
    bp[:, 1:N] = f("b_patch")[:, None] + pos[1:].T
    bp[:, 0] = f("cls_token")[0, 0] + pos[0]
    bp = bp.reshape(DC, 128, NPAD)

    indh = _indh()
    bq_g = f("bqkv_g")[:D].reshape(DC, 128, 1)
    biasblk = (indh * bq_g).astype(BF16)
    pairsel = np.zeros((2, 128), np.float32)
    pairsel[0, 0:64] = 1.0
    pairsel[1, 64:128] = 1.0

    shared = {
        "wpT": wpT,
        "bp": bp,
        "wqkvT_l": f("wqkv_l").T.reshape(DC, 128, E).astype(BF16),
        "woT_l": f("wo_l").T.reshape(DC, 128, D).astype(BF16),
        "wqkvT_g": f("wqkv_g").T.reshape(DC, 128, E).astype(BF16),
        "woT_g": f("wo_g").T.reshape(DC, 128, D).astype(BF16),
        "wclsT": f("w_cls").T.reshape(DC, 128, NCLS).astype(BF16),
        "band01": _band01(),
        "pairsel": pairsel.astype(BF16),
        "ident": np.eye(128, dtype=np.float32).astype(BF16),
        "indh": indh.astype(BF16),
        "biasblk": biasblk,
        "bqkv_l": f("bqkv_l"), "bo_l": f("bo_l"),
        "bqkv_g": f("bqkv_g"), "bo_g": f("bo_g"),
        "g1": f("g1"), "be1": f("be1"), "g2": f("g2"), "be2": f("be2"),
        "b_cls": f("b_cls"),
    }
    in_maps = []
    for c in range(NCORES):
        m = dict(shared)
        m["patchesT"] = patchesT[c * BPC:(c + 1) * BPC]
        in_maps.append(m)
    return in_maps


def kernel(**inputs) -> np.ndarray:
    if "nc" not in _CACHE:
        _CACHE["nc"] = build_nc(debug=False)
    nc = _CACHE["nc"]
    from concourse.bass_utils import run_bass_kernel_spmd
    in_maps = prep_inputs(inputs)
    res = run_bass_kernel_spmd(nc, in_maps, core_ids=list(range(NCORES)))
    return np.concatenate([r["logits"] for r in res.results], axis=0).astype(np.float32)
